# revision 1
# baseline (speedup 1.0000x reference)
# kernel.py — CrystalGCNEncoder (3-layer GAT + global attention pooling) on 8 trn2
# NeuronCores.  Graph-sharded: each core owns 25 graphs' nodes (slots, balanced by
# in-degree over 21 tiles of 128) and all edges whose dst lives there.  Device work
# is split into small SPMD launches; the host only restructures (shard / pad /
# transpose / concat) between launches:
#   P(l):  feat_l = x_l @ [W|W@al] and er_l = x_l @ (W@ar) for own slots (matmuls)
#   L(l):  per-edge gather of feat rows (el bundled in the row tail), edge softmax
#          without max-subtraction (logits are O(1); max cancels exactly), one-hot
#          matmul aggregation in PSUM, normalize + bias + ELU -> x_{l+1}
#   POOL:  gate MLP, per-graph softmax via graph-one-hot matmuls, fp32 latent heads
import numpy as np
import ml_dtypes

N, E, G = 20000, 320000, 200
F_IN, HID, H, LAT = 128, 128, 4, 128
O1, O2, O3 = HID // 2, HID, 2 * HID
D1, D2, D3 = H * O1, H * O2, H * O3          # 256, 512, 1024
NEG_SLOPE = 0.2
NCORES = 8
BF16 = ml_dtypes.bfloat16


def _row_elems(d):          # feat row: [d feats | 4 el | pad] bf16, 256B-multiple
    b = (d + 4) * 2
    return ((b + 255) // 256 * 256) // 2


class Cfg:
    def __init__(self, n, e, g, ntiles, cpt, ncores=NCORES):
        self.n, self.e, self.g, self.ncores = n, e, g, ncores
        self.gpc = g // ncores
        self.ntiles = ntiles
        self.nloc = ntiles * 128
        self.nstar = self.nloc * ncores
        self.cpt = cpt
        self.tpe = cpt * 128
        self.eloc = ntiles * self.tpe
        self.nch = self.eloc // 128
        self.gpad = 32


CFG_FULL = Cfg(N, E, G, ntiles=21, cpt=16)


# ------------------------------------------------------------------ host prep
def host_prep(cfg, node_feat, src, dst, graph_ids):
    n, nc_ = cfg.n, cfg.ncores
    node_feat = np.asarray(node_feat, np.float32)
    src = np.asarray(src).astype(np.int64)
    dst = np.asarray(dst).astype(np.int64)
    graph_ids = np.asarray(graph_ids).astype(np.int64)

    gbounds = np.arange(nc_ + 1) * cfg.gpc
    nbounds = np.searchsorted(graph_ids, gbounds)
    core_of_node = np.searchsorted(nbounds, np.arange(n), side="right") - 1
    indeg = np.bincount(dst, minlength=n)

    glob2slot = np.zeros(n, np.int64)
    tile_of_node = np.zeros(n, np.int64)
    slotpos_of_node = np.zeros(n, np.int64)
    for c in range(nc_):
        nodes = np.arange(nbounds[c], nbounds[c + 1])
        assert len(nodes) <= cfg.nloc
        order = nodes[np.argsort(-indeg[nodes], kind="stable")]
        loads = np.zeros(cfg.ntiles, np.int64)
        counts = np.zeros(cfg.ntiles, np.int64)
        for nd in order:
            free = np.nonzero(counts < 128)[0]
            tgt = free[np.argmin(loads[free])]
            tile_of_node[nd] = tgt
            slotpos_of_node[nd] = counts[tgt]
            glob2slot[nd] = c * cfg.nloc + tgt * 128 + counts[tgt]
            counts[tgt] += 1
            loads[tgt] += indeg[nd]
        assert loads.max() <= cfg.tpe

    edge_core = core_of_node[dst]
    idx32_l, oh_l, oht_l, goh_l = [], [], [], []
    for c in range(nc_):
        eids = np.nonzero(edge_core == c)[0]
        assert len(eids) <= cfg.eloc
        src_slot = np.zeros(cfg.eloc, np.int64)
        dst_pos = np.full(cfg.eloc, -1, np.int64)
        et = tile_of_node[dst[eids]]
        for t in range(cfg.ntiles):
            sel = eids[et == t]
            assert len(sel) <= cfg.tpe
            b = t * cfg.tpe
            src_slot[b : b + len(sel)] = glob2slot[src[sel]]
            dst_pos[b : b + len(sel)] = slotpos_of_node[dst[sel]]
        # per-chunk indices [128, nch] (edge i of chunk ch at [i, ch])
        idx32_l.append(np.ascontiguousarray(
            src_slot.reshape(cfg.nch, 128).T).astype(np.int32))
        oh = np.zeros((cfg.eloc, 128), np.float32)
        v = dst_pos >= 0
        oh[np.nonzero(v)[0], dst_pos[v]] = 1.0
        oh_c = oh.reshape(cfg.nch, 128, 128)
        oh_l.append(oh_c.astype(BF16))
        oht_l.append(np.ascontiguousarray(oh_c.transpose(0, 2, 1)).astype(BF16))
        goh = np.zeros((cfg.ntiles, 128, cfg.gpad), np.float32)
        nodes = np.arange(nbounds[c], nbounds[c + 1])
        goh[tile_of_node[nodes], slotpos_of_node[nodes],
            graph_ids[nodes] - c * cfg.gpc] = 1.0
        goh_l.append(goh.astype(BF16))

    x1 = np.zeros((cfg.nstar, F_IN), np.float32)
    x1[glob2slot] = node_feat
    return dict(glob2slot=glob2slot, nbounds=nbounds, idx32=idx32_l,
                oh=oh_l, oht=oht_l, goh=goh_l, x1=x1)


def fold_weights(W, al, ar):
    Din, D = W.shape
    Hh, O = al.shape
    Wl = np.einsum("iho,ho->ih", W.reshape(Din, Hh, O), al)
    Wr = np.einsum("iho,ho->ih", W.reshape(Din, Hh, O), ar)
    return np.concatenate([W, Wl], 1).astype(np.float32), Wr.astype(np.float32)


def xT_own_blocks(cfg, xblk):
    """[nloc, Din] -> [128, (Din/128)*nloc] with block kc at cols kc*nloc+slot."""
    K = xblk.shape[1] // 128
    return np.ascontiguousarray(
        xblk.reshape(cfg.nloc, K, 128).transpose(2, 1, 0).reshape(128, K * cfg.nloc))


def wstack(Waug):
    """[Din, C] -> [128, Din/128, C] (partition-major K chunks)."""
    Din, C = Waug.shape
    return np.ascontiguousarray(Waug.reshape(Din // 128, 128, C).transpose(1, 0, 2))


def _colchunks(c):
    out, s = [], 0
    while s < c:
        w = min(512, c - s)
        out.append((s, w))
        s += w
    return out


# ------------------------------------------------------------------ builders
def build_P(cfg, Din, Dout):
    import concourse.tile as tile
    from concourse import bacc, mybir

    bf = mybir.dt.bfloat16
    K = Din // 128
    ROW = _row_elems(Dout)
    nc = bacc.Bacc("TRN2", target_bir_lowering=False, debug=False,
                   num_devices=cfg.ncores)
    xT = nc.dram_tensor("xT", [128, K * cfg.nloc], bf, kind="ExternalInput").ap()
    Wa = nc.dram_tensor("Wa", [128, K, Dout + 4], bf, kind="ExternalInput").ap()
    Wr = nc.dram_tensor("Wr", [128, K, 4], bf, kind="ExternalInput").ap()
    feat = nc.dram_tensor("feat", [cfg.nloc, ROW], bf, kind="ExternalOutput").ap()
    er = nc.dram_tensor("er", [cfg.nloc, 4], bf, kind="ExternalOutput").ap()
    cks = _colchunks(Dout + 4)
    with tile.TileContext(nc) as tc:
        with tc.tile_pool(name="w", bufs=1) as wp, \
             tc.tile_pool(name="x", bufs=3) as xp, \
             tc.tile_pool(name="ps", bufs=2, space="PSUM") as pp, \
             tc.tile_pool(name="o", bufs=3) as op:
            Wsb = wp.tile([128, K, Dout + 4], bf)
            nc.sync.dma_start(Wsb[:], Wa[:])
            Wrsb = wp.tile([128, K, 4], bf)
            nc.sync.dma_start(Wrsb[:], Wr[:])
            for t in range(cfg.ntiles):
                pa = [pp.tile([128, w], mybir.dt.float32, tag=f"pa{j}", name=f"pa{j}")
                      for j, (s, w) in enumerate(cks)]
                pe = pp.tile([128, 4], mybir.dt.float32, tag="pe")
                for kc in range(K):
                    xt = xp.tile([128, 128], bf)
                    nc.sync.dma_start(
                        xt[:], xT[:, kc * cfg.nloc + t * 128:
                                  kc * cfg.nloc + (t + 1) * 128])
                    for j, (s, w) in enumerate(cks):
                        nc.tensor.matmul(out=pa[j][:], lhsT=xt[:],
                                         rhs=Wsb[:, kc, s:s + w],
                                         start=(kc == 0), stop=(kc == K - 1))
                    nc.tensor.matmul(out=pe[:], lhsT=xt[:], rhs=Wrsb[:, kc, :],
                                     start=(kc == 0), stop=(kc == K - 1))
                ft = op.tile([128, ROW], bf, tag="ft")
                for j, (s, w) in enumerate(cks):
                    nc.vector.tensor_copy(ft[:, s:s + w], pa[j][:])
                ert = op.tile([128, 4], bf, tag="ert")
                nc.vector.tensor_copy(ert[:], pe[:])
                nc.sync.dma_start(feat[t * 128:(t + 1) * 128, :ROW], ft[:])
                nc.sync.dma_start(er[t * 128:(t + 1) * 128, :], ert[:])
    nc.compile()
    return nc


def build_L(cfg, Dout):
    import concourse.bass as bass
    import concourse.tile as tile
    from concourse import bacc, mybir

    bf = mybir.dt.bfloat16
    f32 = mybir.dt.float32
    ROW = _row_elems(Dout)
    O = Dout // H
    nc = bacc.Bacc("TRN2", target_bir_lowering=False, debug=False,
                   num_devices=cfg.ncores)
    ftab = nc.dram_tensor("ftab", [cfg.nstar, ROW], bf, kind="ExternalInput").ap()
    ero = nc.dram_tensor("ero", [cfg.nloc, 4], bf, kind="ExternalInput").ap()
    idx = nc.dram_tensor("idx", [128, cfg.nch], mybir.dt.int32,
                         kind="ExternalInput").ap()
    OH = nc.dram_tensor("OH", [cfg.nch, 128, 128], bf, kind="ExternalInput").ap()
    OHT = nc.dram_tensor("OHT", [cfg.nch, 128, 128], bf, kind="ExternalInput").ap()
    brow = nc.dram_tensor("brow", [1, Dout], bf, kind="ExternalInput").ap()
    ones1 = nc.dram_tensor("ones1", [1, 128], bf, kind="ExternalInput").ap()
    xn = nc.dram_tensor("xn", [cfg.nloc, Dout], bf, kind="ExternalOutput").ap()
    rcks = _colchunks(Dout)
    with tile.TileContext(nc) as tc:
        with tc.tile_pool(name="c", bufs=1) as cp, \
             tc.tile_pool(name="g", bufs=2 * cfg.cpt + 2) as gp, \
             tc.tile_pool(name="oh", bufs=4) as ohp, \
             tc.tile_pool(name="s", bufs=2) as sp, \
             tc.tile_pool(name="ps", bufs=2, space="PSUM") as pp:
            idxsb = cp.tile([128, cfg.nch], mybir.dt.int32)
            nc.sync.dma_start(idxsb[:], idx[:])
            ersb = cp.tile([128, cfg.ntiles * 4], bf)
            for t in range(cfg.ntiles):
                nc.sync.dma_start(ersb[:, t * 4:(t + 1) * 4],
                                  ero[t * 128:(t + 1) * 128, :])
            on1 = cp.tile([1, 128], bf)
            nc.sync.dma_start(on1[:], ones1[:])
            brsb = cp.tile([1, Dout], bf)
            nc.sync.dma_start(brsb[:], brow[:])
            bps = pp.tile([128, Dout], f32, tag="bias", bufs=1)
            for (s, w) in rcks:
                nc.tensor.matmul(out=bps[:, s:s + w], lhsT=on1[:],
                                 rhs=brsb[:, s:s + w], start=True, stop=True)
            bsb = cp.tile([128, Dout], f32)
            nc.vector.tensor_copy(bsb[:], bps[:])
            for t in range(cfg.ntiles):
                gts = []
                erps = pp.tile([128, 64], f32, tag="erps")
                for c in range(cfg.cpt):
                    ch = t * cfg.cpt + c
                    gt = gp.tile([128, ROW], bf, tag="g")
                    nc.gpsimd.indirect_dma_start(
                        out=gt[:], out_offset=None, in_=ftab[:],
                        in_offset=bass.IndirectOffsetOnAxis(
                            ap=idxsb[:, ch:ch + 1], axis=0))
                    gts.append(gt)
                    oht = ohp.tile([128, 128], bf, tag="oht")
                    nc.sync.dma_start(oht[:], OHT[ch])
                    nc.tensor.matmul(out=erps[:, c * 4:(c + 1) * 4], lhsT=oht[:],
                                     rhs=ersb[:, t * 4:(t + 1) * 4],
                                     start=True, stop=True)
                zz = sp.tile([128, 64], f32, tag="zz")
                for c in range(cfg.cpt):
                    nc.vector.tensor_add(zz[:, c * 4:(c + 1) * 4],
                                         gts[c][:, Dout:Dout + 4],
                                         erps[:, c * 4:(c + 1) * 4])
                za = sp.tile([128, 64], f32, tag="za")
                nc.vector.scalar_tensor_tensor(
                    out=za[:], in0=zz[:], scalar=NEG_SLOPE, in1=zz[:],
                    op0=mybir.AluOpType.mult, op1=mybir.AluOpType.max)
                ee = sp.tile([128, 64], bf, tag="ee")
                nc.scalar.activation(ee[:], za[:],
                                     mybir.ActivationFunctionType.Exp)
                denps = pp.tile([128, 4], f32, tag="den")
                rstps = [pp.tile([128, w], f32, tag=f"rst{j}", name=f"rst{j}", bufs=1)
                         for j, (s, w) in enumerate(rcks)]
                for c in range(cfg.cpt):
                    gt = gts[c]
                    for h in range(H):
                        nc.vector.scalar_tensor_tensor(
                            out=gt[:, h * O:(h + 1) * O],
                            in0=gt[:, h * O:(h + 1) * O], scalar=1.0,
                            in1=ee[:, c * 4 + h:c * 4 + h + 1].to_broadcast(
                                [128, O]),
                            op0=mybir.AluOpType.mult, op1=mybir.AluOpType.mult)
                    ohc = ohp.tile([128, 128], bf, tag="ohc")
                    nc.sync.dma_start(ohc[:], OH[t * cfg.cpt + c])
                    nc.tensor.matmul(out=denps[:], lhsT=ohc[:],
                                     rhs=ee[:, c * 4:(c + 1) * 4],
                                     start=(c == 0), stop=(c == cfg.cpt - 1))
                    for j, (s, w) in enumerate(rcks):
                        nc.tensor.matmul(out=rstps[j][:], lhsT=ohc[:],
                                         rhs=gt[:, s:s + w],
                                         start=(c == 0), stop=(c == cfg.cpt - 1))
                dcl = sp.tile([128, 4], f32, tag="dcl")
                nc.vector.tensor_scalar_max(dcl[:], denps[:], 1e-9)
                rec = sp.tile([128, 4], f32, tag="rec")
                nc.vector.reciprocal(rec[:], dcl[:])
                y = sp.tile([128, Dout], f32, tag="y")
                for h in range(H):
                    j = (h * O) // 512
                    s0 = (h * O) % 512
                    nc.vector.scalar_tensor_tensor(
                        out=y[:, h * O:(h + 1) * O], in0=rstps[j][:, s0:s0 + O],
                        scalar=rec[:, h:h + 1], in1=bsb[:, h * O:(h + 1) * O],
                        op0=mybir.AluOpType.mult, op1=mybir.AluOpType.add)
                mn = sp.tile([128, Dout], f32, tag="mn")
                nc.vector.tensor_scalar_min(mn[:], y[:], 0.0)
                ex = sp.tile([128, Dout], f32, tag="ex")
                nc.scalar.activation(ex[:], mn[:],
                                     mybir.ActivationFunctionType.Exp)
                y2 = sp.tile([128, Dout], f32, tag="y2")
                nc.vector.scalar_tensor_tensor(
                    out=y2[:], in0=y[:], scalar=0.0, in1=ex[:],
                    op0=mybir.AluOpType.max, op1=mybir.AluOpType.add)
                xo = sp.tile([128, Dout], bf, tag="xo")
                nc.vector.tensor_scalar_add(xo[:], y2[:], -1.0)
                nc.sync.dma_start(xn[t * 128:(t + 1) * 128, :], xo[:])
    nc.compile()
    return nc


def build_POOL(cfg):
    import concourse.tile as tile
    from concourse import bacc, mybir

    bf = mybir.dt.bfloat16
    f32 = mybir.dt.float32
    nc = bacc.Bacc("TRN2", target_bir_lowering=False, debug=False,
                   num_devices=cfg.ncores)
    h3T = nc.dram_tensor("h3T", [128, 8 * cfg.nloc], bf, kind="ExternalInput").ap()
    h3 = nc.dram_tensor("h3", [cfg.nloc, D3], bf, kind="ExternalInput").ap()
    Wg1 = nc.dram_tensor("Wg1", [128, 8, 128], bf, kind="ExternalInput").ap()
    bg1c = nc.dram_tensor("bg1c", [128, 1], f32, kind="ExternalInput").ap()
    Wg2c = nc.dram_tensor("Wg2c", [128, 1], bf, kind="ExternalInput").ap()
    bg2r = nc.dram_tensor("bg2r", [128, 1], f32, kind="ExternalInput").ap()
    GOH = nc.dram_tensor("GOH", [cfg.ntiles, 128, cfg.gpad], bf,
                         kind="ExternalInput").ap()
    Wmu = nc.dram_tensor("Wmu", [128, 8, 128], f32, kind="ExternalInput").ap()
    Wlv = nc.dram_tensor("Wlv", [128, 8, 128], f32, kind="ExternalInput").ap()
    bmu = nc.dram_tensor("bmu", [1, 128], f32, kind="ExternalInput").ap()
    blv = nc.dram_tensor("blv", [1, 128], f32, kind="ExternalInput").ap()
    on32 = nc.dram_tensor("on32", [1, 32], f32, kind="ExternalInput").ap()
    identd = nc.dram_tensor("identd", [32, 32], f32, kind="ExternalInput").ap()
    mu = nc.dram_tensor("mu", [cfg.gpad, 128], f32, kind="ExternalOutput").ap()
    lv = nc.dram_tensor("lv", [cfg.gpad, 128], f32, kind="ExternalOutput").ap()
    nwin = (cfg.nloc + 511) // 512
    with tile.TileContext(nc) as tc:
        with tc.tile_pool(name="c", bufs=1) as cp, \
             tc.tile_pool(name="s", bufs=3) as sp, \
             tc.tile_pool(name="ps", bufs=1, space="PSUM") as pp:
            Wg1s = cp.tile([128, 8, 128], bf)
            nc.sync.dma_start(Wg1s[:], Wg1[:])
            h3Ts = cp.tile([128, 8 * cfg.nloc], bf)
            nc.sync.dma_start(h3Ts[:], h3T[:])
            small = {}
            for nm, ap_, dt_ in [("bg1c", bg1c, f32), ("Wg2c", Wg2c, bf),
                                 ("bg2r", bg2r, f32), ("on32", on32, f32),
                                 ("bmu", bmu, f32), ("blv", blv, f32)]:
                tl = cp.tile(list(ap_.shape), dt_, tag=nm, name=nm)
                nc.sync.dma_start(tl[:], ap_[:])
                small[nm] = tl
            GOHs = cp.tile([128, cfg.ntiles * cfg.gpad], bf)
            for t in range(cfg.ntiles):
                nc.sync.dma_start(GOHs[:, t * cfg.gpad:(t + 1) * cfg.gpad],
                                  GOH[t])
            relu1 = cp.tile([128, cfg.nloc], bf)
            for w in range(nwin):
                s = w * 512
                ww = min(512, cfg.nloc - s)
                ps = pp.tile([128, 512], f32, tag="g1")
                for kc in range(8):
                    nc.tensor.matmul(out=ps[:, :ww], lhsT=Wg1s[:, kc, :],
                                     rhs=h3Ts[:, kc * cfg.nloc + s:
                                              kc * cfg.nloc + s + ww],
                                     start=(kc == 0), stop=(kc == 7))
                nc.scalar.activation(relu1[:, s:s + ww], ps[:, :ww],
                                     mybir.ActivationFunctionType.Relu,
                                     bias=small["bg1c"][:])
            gps = pp.tile([128, 32], f32, tag="g2")
            for t in range(cfg.ntiles):
                nc.tensor.matmul(out=gps[:, t:t + 1],
                                 lhsT=relu1[:, t * 128:(t + 1) * 128],
                                 rhs=small["Wg2c"][:], start=True, stop=True)
            eg = sp.tile([128, cfg.ntiles], bf, tag="eg")
            nc.scalar.activation(eg[:], gps[:, :cfg.ntiles],
                                 mybir.ActivationFunctionType.Exp,
                                 bias=small["bg2r"][:])
            gd = pp.tile([cfg.gpad, 1], f32, tag="gd")
            goha = sp.tile([128, cfg.ntiles * cfg.gpad], bf, tag="goha")
            for t in range(cfg.ntiles):
                nc.tensor.matmul(out=gd[:], lhsT=GOHs[:, t * cfg.gpad:
                                                      (t + 1) * cfg.gpad],
                                 rhs=eg[:, t:t + 1],
                                 start=(t == 0), stop=(t == cfg.ntiles - 1))
                nc.vector.tensor_mul(
                    goha[:, t * cfg.gpad:(t + 1) * cfg.gpad],
                    GOHs[:, t * cfg.gpad:(t + 1) * cfg.gpad],
                    eg[:, t:t + 1].to_broadcast([128, cfg.gpad]))
            h3s = sp.tile([128, D3], bf, tag="h3s")
            geps = [pp.tile([cfg.gpad, 512], f32, tag=f"ge{j}", name=f"geps{j}") for j in range(2)]
            for t in range(cfg.ntiles):
                h3t = sp.tile([128, D3], bf, tag="h3t")
                nc.sync.dma_start(h3t[:], h3[t * 128:(t + 1) * 128, :])
                for j in range(2):
                    nc.tensor.matmul(out=geps[j][:],
                                     lhsT=goha[:, t * cfg.gpad:(t + 1) * cfg.gpad],
                                     rhs=h3t[:, j * 512:(j + 1) * 512],
                                     start=(t == 0), stop=(t == cfg.ntiles - 1))
            gdc = sp.tile([cfg.gpad, 1], f32, tag="gdc")
            nc.vector.tensor_scalar_max(gdc[:], gd[:], 1e-9)
            grc = sp.tile([cfg.gpad, 1], f32, tag="grc")
            nc.vector.reciprocal(grc[:], gdc[:])
            zge = sp.tile([cfg.gpad, D3], f32, tag="zge")
            nc.vector.memset(zge[:], 0.0)
            ge = sp.tile([cfg.gpad, D3], f32, tag="ge")
            for j in range(2):
                nc.vector.scalar_tensor_tensor(
                    out=ge[:, j * 512:(j + 1) * 512], in0=geps[j][:],
                    scalar=grc[:, 0:1], in1=zge[:, j * 512:(j + 1) * 512],
                    op0=mybir.AluOpType.mult, op1=mybir.AluOpType.add)
            # transpose ge via PE (fp32): [gpad,128]-chunks -> geT [128, 8*gpad]
            if True:
                ident = cp.tile([cfg.gpad, cfg.gpad], f32, tag="ident")
                nc.sync.dma_start(ident[:], identd[:])
                geT = sp.tile([128, 8 * cfg.gpad], f32, tag="geT")
                for kc in range(8):
                    pst = pp.tile([128, cfg.gpad], f32, tag="pst")
                    nc.tensor.transpose(out=pst[:],
                                        in_=ge[:, kc * 128:(kc + 1) * 128],
                                        identity=ident[:])
                    nc.vector.tensor_copy(geT[:, kc * cfg.gpad:(kc + 1) * cfg.gpad],
                                          pst[:])
                for nm, Wt, bt, outp in [("mu", Wmu, "bmu", mu),
                                         ("lv", Wlv, "blv", lv)]:
                    Ws = sp.tile([128, 8, 128], f32, tag="Wmlv")
                    nc.sync.dma_start(Ws[:], Wt[:])
                    mps = pp.tile([cfg.gpad, 128], f32, tag="mps")
                    for kc in range(8):
                        nc.tensor.matmul(out=mps[:],
                                         lhsT=geT[:, kc * cfg.gpad:(kc + 1) * cfg.gpad],
                                         rhs=Ws[:, kc, :],
                                         start=(kc == 0), stop=False)
                    nc.tensor.matmul(out=mps[:], lhsT=small["on32"][:],
                                     rhs=small[bt][:],
                                     start=False, stop=True)
                    mo = sp.tile([cfg.gpad, 128], f32, tag="mo")
                    nc.vector.tensor_copy(mo[:], mps[:])
                    nc.sync.dma_start(outp[:], mo[:])
    nc.compile()
    return nc


_BUILD_CACHE = {}


def _get(key, fn):
    if key not in _BUILD_CACHE:
        _BUILD_CACHE[key] = fn()
    return _BUILD_CACHE[key]


def _run(nc, in_maps):
    from concourse.bass_utils import run_bass_kernel_spmd
    return run_bass_kernel_spmd(nc, in_maps, core_ids=list(range(NCORES))).results


# ------------------------------------------------------------------ main entry
def kernel(node_feat, src, dst, graph_ids,
           W1, al1, ar1, b1, W2, al2, ar2, b2, W3, al3, ar3, b3,
           Wg1, bg1, Wg2, bg2, Wmu, bmu, Wlv, blv, cfg=None):
    cfg = cfg or CFG_FULL
    nc_ = cfg.ncores
    prep = host_prep(cfg, node_feat, src, dst, graph_ids)
    layers = [(np.asarray(W1, np.float32), np.asarray(al1, np.float32),
               np.asarray(ar1, np.float32), np.asarray(b1, np.float32)),
              (np.asarray(W2, np.float32), np.asarray(al2, np.float32),
               np.asarray(ar2, np.float32), np.asarray(b2, np.float32)),
              (np.asarray(W3, np.float32), np.asarray(al3, np.float32),
               np.asarray(ar3, np.float32), np.asarray(b3, np.float32))]
    douts = [D1, D2, D3]

    xblocks = [np.ascontiguousarray(prep["x1"][c * cfg.nloc:(c + 1) * cfg.nloc])
               for c in range(nc_)]
    for li, (W, al, ar, b) in enumerate(layers):
        Din, Dout = W.shape
        ROW = _row_elems(Dout)
        Waug, Wr = fold_weights(W, al, ar)
        ncP = _get(("P", Din, Dout), lambda: build_P(cfg, Din, Dout))
        inP = [dict(xT=xT_own_blocks(cfg, xblocks[c]).astype(BF16),
                    Wa=wstack(Waug).astype(BF16), Wr=wstack(Wr).astype(BF16))
               for c in range(nc_)]
        outP = _run(ncP, inP)
        ftab = np.concatenate([outP[c]["feat"] for c in range(nc_)], 0)
        ncL = _get(("L", Dout), lambda: build_L(cfg, Dout))
        inL = [dict(ftab=ftab, ero=outP[c]["er"], idx=prep["idx32"][c],
                    OH=prep["oh"][c], OHT=prep["oht"][c],
                    brow=b[None].astype(BF16),
                    ones1=np.ones((1, 128), BF16))
               for c in range(nc_)]
        outL = _run(ncL, inL)
        xblocks = [outL[c]["xn"].astype(np.float32) for c in range(nc_)]

    ncPool = _get(("POOL",), lambda: build_POOL(cfg))
    Wg1f = np.asarray(Wg1, np.float32)
    inPool = [dict(
        h3T=xT_own_blocks(cfg, xblocks[c]).astype(BF16),
        h3=xblocks[c].astype(BF16),
        Wg1=wstack(Wg1f).astype(BF16),
        bg1c=np.asarray(bg1, np.float32).reshape(128, 1),
        Wg2c=np.asarray(Wg2, BF16).reshape(128, 1),
        bg2r=np.full((128, 1), np.asarray(bg2, np.float32).reshape(-1)[0],
                     np.float32),
        GOH=prep["goh"][c],
        Wmu=wstack(np.asarray(Wmu, np.float32)),
        Wlv=wstack(np.asarray(Wlv, np.float32)),
        bmu=np.asarray(bmu, np.float32)[None],
        blv=np.asarray(blv, np.float32)[None],
        on32=np.ones((1, 32), np.float32),
        identd=np.eye(32, dtype=np.float32)) for c in range(nc_)]
    outPool = _run(ncPool, inPool)
    mu = np.concatenate([outPool[c]["mu"][:cfg.gpc] for c in range(nc_)], 0)
    lv = np.concatenate([outPool[c]["lv"][:cfg.gpc] for c in range(nc_)], 0)
    return np.asarray(mu, np.float32), np.asarray(lv, np.float32)



# revision 2
# speedup vs baseline: 2.1797x; 2.1797x over previous
# kernel_fused.py — CrystalGCNEncoder (3-layer GAT + global attention pooling) on
# 8 trn2 NeuronCores, fused into ONE SPMD launch.
#
# vs the 7-launch baseline: the inter-layer feature-table exchange is an on-device
# AllGather (DRAM->Shared DRAM), the one-hot scatter/gather matrices are built on
# device from int32 slot indices (iota + is_equal), the big weight matrices are
# uploaded partition-sharded and AllGathered on device, and the inter-layer
# transposes run on the PE (identity matmul).  Host->device upload drops from
# ~1.2GB across 7 launches to ~11MB in one launch.
import numpy as np
import ml_dtypes

N, E, G = 20000, 320000, 200
F_IN, HID, H, LAT = 128, 128, 4, 128
O1, O2, O3 = HID // 2, HID, 2 * HID
D1, D2, D3 = H * O1, H * O2, H * O3          # 256, 512, 1024
NEG_SLOPE = 0.2
NCORES = 8
BF16 = ml_dtypes.bfloat16


def _row_elems(d):          # feat row: [d feats | 4 el | pad] bf16, 256B-multiple
    b = (d + 4) * 2
    return ((b + 255) // 256 * 256) // 2


class Cfg:
    def __init__(self, n, e, g, ntiles, cpt, ncores=NCORES):
        self.n, self.e, self.g, self.ncores = n, e, g, ncores
        self.gpc = g // ncores
        self.ntiles = ntiles
        self.nloc = ntiles * 128
        self.nstar = self.nloc * ncores
        self.cpt = cpt
        self.tpe = cpt * 128
        self.eloc = ntiles * self.tpe
        self.nch = self.eloc // 128
        self.gpad = 32


CFG_FULL = Cfg(N, E, G, ntiles=21, cpt=16)

LAYERS_DIMS = [(F_IN, D1), (D1, D2), (D2, D3)]

# ---------------------------------------------------------------- weight packing
# wbmat: [128, WBCOLS] bf16, partition-sharded 16 rows/core, AllGathered on device
# wfmat: [128, WFCOLS] f32, same
# smalls_b: [1, SB] bf16 replicated;  smalls_f: [1, SF] f32 replicated
def _seg_layout():
    wb, wf, sb, sf = {}, {}, {}, {}
    ob = of = osb = osf = 0

    def addb(name, cols):
        nonlocal ob
        wb[name] = (ob, cols)
        ob += cols

    def addf(name, cols):
        nonlocal of
        wf[name] = (of, cols)
        of += cols

    def addsb(name, cols):
        nonlocal osb
        sb[name] = (osb, cols)
        osb += cols

    def addsf(name, cols):
        nonlocal osf
        sf[name] = (osf, cols)
        osf += cols

    addb("W1a", 1 * (D1 + 4)); addb("W2a", 2 * (D2 + 4)); addb("W3a", 4 * (D3 + 4))
    addb("Wr1", 1 * 4); addb("Wr2", 2 * 4); addb("Wr3", 4 * 4)
    addb("Wg1", 8 * 128)
    addf("Wmu", 8 * 128); addf("Wlv", 8 * 128)
    addsb("b1", D1); addsb("b2", D2); addsb("b3", D3); addsb("Wg2", 128)
    addsf("bg1", 128); addsf("bmu", 128); addsf("blv", 128); addsf("bg2", 1)
    return wb, ob, wf, of, sb, osb, sf, osf


SEG_B, WBCOLS, SEG_F, WFCOLS, SEG_SB, SBCOLS, SEG_SF, SFCOLS = _seg_layout()


def fold_weights(W, al, ar):
    Din, D = W.shape
    Hh, O = al.shape
    Wl = np.einsum("iho,ho->ih", W.reshape(Din, Hh, O), al)
    Wr = np.einsum("iho,ho->ih", W.reshape(Din, Hh, O), ar)
    return np.concatenate([W, Wl], 1).astype(np.float32), Wr.astype(np.float32)


def wstack(Waug):
    """[Din, C] -> [128, Din/128, C] (partition-major K chunks)."""
    Din, C = Waug.shape
    return np.ascontiguousarray(Waug.reshape(Din // 128, 128, C).transpose(1, 0, 2))


def _colchunks(c):
    out, s = [], 0
    while s < c:
        w = min(512, c - s)
        out.append((s, w))
        s += w
    return out


# ------------------------------------------------------------------ host prep
def host_prep(cfg, node_feat, src, dst, graph_ids):
    n, nc_ = cfg.n, cfg.ncores
    node_feat = np.asarray(node_feat, np.float32)
    src = np.asarray(src).astype(np.int64)
    dst = np.asarray(dst).astype(np.int64)
    graph_ids = np.asarray(graph_ids).astype(np.int64)

    gbounds = np.arange(nc_ + 1) * cfg.gpc
    nbounds = np.searchsorted(graph_ids, gbounds)
    core_of_node = np.searchsorted(nbounds, np.arange(n), side="right") - 1
    indeg = np.bincount(dst, minlength=n)

    glob2slot = np.zeros(n, np.int64)
    tile_of_node = np.zeros(n, np.int64)
    slotpos_of_node = np.zeros(n, np.int64)
    for c in range(nc_):
        nodes = np.arange(nbounds[c], nbounds[c + 1])
        assert len(nodes) <= cfg.nloc
        order = nodes[np.argsort(-indeg[nodes], kind="stable")]
        loads = np.zeros(cfg.ntiles, np.int64)
        counts = np.zeros(cfg.ntiles, np.int64)
        for nd in order:
            free = np.nonzero(counts < 128)[0]
            tgt = free[np.argmin(loads[free])]
            tile_of_node[nd] = tgt
            slotpos_of_node[nd] = counts[tgt]
            glob2slot[nd] = c * cfg.nloc + tgt * 128 + counts[tgt]
            counts[tgt] += 1
            loads[tgt] += indeg[nd]
        assert loads.max() <= cfg.tpe

    edge_core = core_of_node[dst]
    idx32_l, dpos_l, gid_l = [], [], []
    for c in range(nc_):
        eids = np.nonzero(edge_core == c)[0]
        assert len(eids) <= cfg.eloc
        src_slot = np.zeros(cfg.eloc, np.int64)
        dst_pos = np.full(cfg.eloc, -1, np.int64)
        et = tile_of_node[dst[eids]]
        for t in range(cfg.ntiles):
            sel = eids[et == t]
            assert len(sel) <= cfg.tpe
            b = t * cfg.tpe
            src_slot[b : b + len(sel)] = glob2slot[src[sel]]
            dst_pos[b : b + len(sel)] = slotpos_of_node[dst[sel]]
        # per-chunk indices [128, nch] (edge i of chunk ch at [i, ch])
        idx32_l.append(np.ascontiguousarray(
            src_slot.reshape(cfg.nch, 128).T).astype(np.int32))
        dpos_l.append(np.ascontiguousarray(
            dst_pos.reshape(cfg.nch, 128).T).astype(np.float32))
        # local graph id per slot [128, ntiles] (-1 for pad slots)
        gid = np.full((cfg.ntiles, 128), -1, np.int64)
        nodes = np.arange(nbounds[c], nbounds[c + 1])
        gid[tile_of_node[nodes], slotpos_of_node[nodes]] = (
            graph_ids[nodes] - c * cfg.gpc)
        gid_l.append(np.ascontiguousarray(gid.T).astype(np.float32))

    x1 = np.zeros((cfg.nstar, F_IN), np.float32)
    x1[glob2slot] = node_feat
    # per-core transposed feature block [128, nloc]
    x1T_l = [np.ascontiguousarray(
        x1[c * cfg.nloc:(c + 1) * cfg.nloc].T).astype(BF16)
        for c in range(nc_)]
    return dict(idx32=idx32_l, dpos=dpos_l, gid=gid_l, x1T=x1T_l)


# ------------------------------------------------------------------ the kernel
def build_fused(cfg):
    import concourse.bass as bass
    import concourse.tile as tile
    from concourse import bacc, mybir

    bf = mybir.dt.bfloat16
    f32 = mybir.dt.float32
    i32 = mybir.dt.int32
    AF = mybir.ActivationFunctionType
    ALU = mybir.AluOpType
    RG = [list(range(cfg.ncores))]
    SHARD = 128 // cfg.ncores

    nc = bacc.Bacc("TRN2", target_bir_lowering=False, debug=False,
                   num_devices=cfg.ncores)
    x1T_in = nc.dram_tensor("x1T", [128, cfg.nloc], bf, kind="ExternalInput").ap()
    idx_in = nc.dram_tensor("idx", [128, cfg.nch], i32, kind="ExternalInput").ap()
    dp_in = nc.dram_tensor("dpos", [128, cfg.nch], f32, kind="ExternalInput").ap()
    gid_in = nc.dram_tensor("gid", [128, cfg.ntiles], f32,
                            kind="ExternalInput").ap()
    wb_in = nc.dram_tensor("wb", [SHARD, WBCOLS], bf, kind="ExternalInput").ap()
    wf_in = nc.dram_tensor("wf", [SHARD, WFCOLS], f32, kind="ExternalInput").ap()
    sb_in = nc.dram_tensor("smb", [1, SBCOLS], bf, kind="ExternalInput").ap()
    sf_in = nc.dram_tensor("smf", [1, SFCOLS], f32, kind="ExternalInput").ap()
    mu_out = nc.dram_tensor("mu", [cfg.gpad, 128], f32, kind="ExternalOutput").ap()
    lv_out = nc.dram_tensor("lv", [cfg.gpad, 128], f32, kind="ExternalOutput").ap()

    with tile.TileContext(nc) as tc:
        with tc.tile_pool(name="dram", bufs=1, space="DRAM") as dpool, \
             tc.tile_pool(name="glob", bufs=1) as gl:
            # ---- distribute weights: partition-sharded upload + AllGather
            wbb = dpool.tile([SHARD, WBCOLS], bf, tag="wbb", name="wbb")
            wfb = dpool.tile([SHARD, WFCOLS], f32, tag="wfb", name="wfb")
            nc.sync.dma_start(wbb[:], wb_in[:])
            nc.sync.dma_start(wfb[:], wf_in[:])
            wbfull = dpool.tile([128, WBCOLS], bf, tag="wbfull", name="wbfull",
                                addr_space="Shared")
            wffull = dpool.tile([128, WFCOLS], f32, tag="wffull", name="wffull",
                                addr_space="Shared")
            nc.gpsimd.collective_compute(
                "AllGather", ALU.bypass, replica_groups=RG,
                ins=[wbb.opt()], outs=[wbfull.opt()])
            nc.gpsimd.collective_compute(
                "AllGather", ALU.bypass, replica_groups=RG,
                ins=[wfb.opt()], outs=[wffull.opt()])

            W = {}
            for nm, K, C in [("W1a", 1, D1 + 4), ("W2a", 2, D2 + 4),
                             ("W3a", 4, D3 + 4), ("Wr1", 1, 4), ("Wr2", 2, 4),
                             ("Wr3", 4, 4), ("Wg1", 8, 128)]:
                o, ncols = SEG_B[nm]
                t = gl.tile([128, K, C], bf, tag=nm, name=nm)
                nc.sync.dma_start(t[:], wbfull[:, o:o + ncols])
                W[nm] = t
            for nm in ["Wmu", "Wlv"]:
                o, ncols = SEG_F[nm]
                t = gl.tile([128, 8, 128], f32, tag=nm, name=nm)
                nc.sync.dma_start(t[:], wffull[:, o:o + ncols])
                W[nm] = t
            for nm, dt_ in [("b1", bf), ("b2", bf), ("b3", bf)]:
                o, ncols = SEG_SB[nm]
                t = gl.tile([1, ncols], dt_, tag=nm, name=nm)
                nc.sync.dma_start(t[:], sb_in[0:1, o:o + ncols])
                W[nm] = t
            o, ncols = SEG_SB["Wg2"]
            Wg2c = gl.tile([128, 1], bf, tag="Wg2", name="Wg2c")
            nc.sync.dma_start(Wg2c[:],
                              sb_in[0:1, o:o + 128].rearrange("a b -> b a"))
            bg1c = gl.tile([128, 1], f32, tag="bg1", name="bg1c")
            o, _ = SEG_SF["bg1"]
            nc.sync.dma_start(bg1c[:],
                              sf_in[0:1, o:o + 128].rearrange("a b -> b a"))
            for nm in ["bmu", "blv"]:
                o, ncols = SEG_SF[nm]
                t = gl.tile([1, 128], f32, tag=nm, name=nm)
                nc.sync.dma_start(t[:], sf_in[0:1, o:o + ncols])
                W[nm] = t
            bg2r = gl.tile([128, 1], f32, tag="bg2", name="bg2r")
            o, _ = SEG_SF["bg2"]
            nc.sync.dma_start(bg2r[:],
                              sf_in[0:1, o:o + 1].to_broadcast([128, 1]))

            # ---- constants
            iotaF = gl.tile([128, 128], f32, tag="iotaF", name="iotaF")
            nc.gpsimd.iota(iotaF[:], pattern=[[1, 128]], base=0,
                           channel_multiplier=0,
                           allow_small_or_imprecise_dtypes=True)
            iotaC = gl.tile([128, 1], f32, tag="iotaC", name="iotaC")
            nc.gpsimd.iota(iotaC[:], pattern=[[0, 1]], base=0,
                           channel_multiplier=1,
                           allow_small_or_imprecise_dtypes=True)
            identb = gl.tile([128, 128], bf, tag="identb", name="identb")
            nc.vector.tensor_scalar(out=identb[:], in0=iotaF[:],
                                    scalar1=iotaC[:, 0:1], scalar2=None,
                                    op0=ALU.is_equal)
            iota32 = gl.tile([128, 32], f32, tag="iota32", name="iota32")
            nc.gpsimd.iota(iota32[:], pattern=[[1, 32]], base=0,
                           channel_multiplier=0,
                           allow_small_or_imprecise_dtypes=True)
            ident32 = gl.tile([32, 32], f32, tag="ident32", name="ident32")
            nc.vector.tensor_scalar(out=ident32[:], in0=iota32[0:32, :],
                                    scalar1=iotaC[0:32, 0:1], scalar2=None,
                                    op0=ALU.is_equal)
            onesr = gl.tile([1, 128], bf, tag="onesr", name="onesr")
            nc.vector.memset(onesr[:], 1.0)
            on32 = gl.tile([1, 32], f32, tag="on32", name="on32")
            nc.vector.memset(on32[:], 1.0)

            idxsb = gl.tile([128, cfg.nch], i32, tag="idxsb", name="idxsb")
            nc.sync.dma_start(idxsb[:], idx_in[:])
            dpsb = gl.tile([128, cfg.nch], f32, tag="dpsb", name="dpsb")
            nc.sync.dma_start(dpsb[:], dp_in[:])
            gidsb = gl.tile([128, cfg.ntiles], f32, tag="gidsb", name="gidsb")
            nc.sync.dma_start(gidsb[:], gid_in[:])
            GOHs = gl.tile([128, cfg.ntiles * cfg.gpad], bf, tag="GOHs",
                           name="GOHs")
            for t in range(cfg.ntiles):
                nc.vector.tensor_scalar(
                    out=GOHs[:, t * cfg.gpad:(t + 1) * cfg.gpad],
                    in0=iota32[:], scalar1=gidsb[:, t:t + 1], scalar2=None,
                    op0=ALU.is_equal)
            ersb = gl.tile([128, cfg.ntiles * 4], bf, tag="ersb", name="ersb")

            # DRAM intermediates
            ftabs, fwr = [], []
            for li, (Din, Dout) in enumerate(LAYERS_DIMS):
                ROW = _row_elems(Dout)
                fwr.append(dpool.tile([cfg.nloc, ROW], bf, tag=f"fw{li}",
                                      name=f"fw{li}"))
                ftabs.append(dpool.tile([cfg.nstar, ROW], bf, tag=f"ft{li}",
                                        name=f"ft{li}", addr_space="Shared"))
            xTd = [None]
            for li, (Din, Dout) in enumerate(LAYERS_DIMS[1:] + [(D3, 0)]):
                xTd.append(dpool.tile([128, (Din // 128) * cfg.nloc], bf,
                                      tag=f"xT{li + 1}", name=f"xT{li + 1}"))
            h3d = dpool.tile([cfg.nloc, D3], bf, tag="h3d", name="h3d")

            # ================= three GAT layers =================
            for li, (Din, Dout) in enumerate(LAYERS_DIMS):
                K = Din // 128
                ROW = _row_elems(Dout)
                O = Dout // H
                Wa, Wr = W[f"W{li + 1}a"], W[f"Wr{li + 1}"]
                brow = W[f"b{li + 1}"]
                cks = _colchunks(Dout + 4)
                rcks = _colchunks(Dout)
                xsrc = x1T_in if li == 0 else xTd[li]

                # ---------- P: feat = x @ [W|W@al], er = x @ (W@ar)
                with tc.tile_pool(name=f"px{li}", bufs=3) as xp, \
                     tc.tile_pool(name=f"pp{li}", bufs=2, space="PSUM") as pp, \
                     tc.tile_pool(name=f"po{li}", bufs=3) as op:
                    for t in range(cfg.ntiles):
                        pa = [pp.tile([128, w], f32, tag=f"pa{j}", name=f"pa{j}")
                              for j, (s, w) in enumerate(cks)]
                        pe = pp.tile([128, 4], f32, tag="pe", name="pe")
                        for kc in range(K):
                            xt = xp.tile([128, 128], bf, tag="xt", name="xt")
                            nc.sync.dma_start(
                                xt[:], xsrc[:, kc * cfg.nloc + t * 128:
                                            kc * cfg.nloc + (t + 1) * 128])
                            for j, (s, w) in enumerate(cks):
                                nc.tensor.matmul(out=pa[j][:], lhsT=xt[:],
                                                 rhs=Wa[:, kc, s:s + w],
                                                 start=(kc == 0),
                                                 stop=(kc == K - 1))
                            nc.tensor.matmul(out=pe[:], lhsT=xt[:],
                                             rhs=Wr[:, kc, :],
                                             start=(kc == 0), stop=(kc == K - 1))
                        ft = op.tile([128, ROW], bf, tag="ft", name="ft")
                        for j, (s, w) in enumerate(cks):
                            nc.vector.tensor_copy(ft[:, s:s + w], pa[j][:])
                        nc.vector.tensor_copy(ersb[:, t * 4:(t + 1) * 4], pe[:])
                        nc.sync.dma_start(fwr[li][t * 128:(t + 1) * 128, :ROW],
                                          ft[:])

                # ---------- exchange feature tables
                nc.gpsimd.collective_compute(
                    "AllGather", ALU.bypass, replica_groups=RG,
                    ins=[fwr[li].opt()], outs=[ftabs[li].opt()])
                ftab = ftabs[li]

                # ---------- L: gather, edge softmax, aggregate, ELU
                with tc.tile_pool(name=f"lb{li}", bufs=1,
                                  space="PSUM") as bpp:
                    bps = bpp.tile([128, Dout], f32, tag="bias", name="bps")
                    for (s, w) in rcks:
                        nc.tensor.matmul(out=bps[:, s:s + w], lhsT=onesr[:],
                                         rhs=brow[:, s:s + w],
                                         start=True, stop=True)
                    bsb = gl.tile([128, Dout], f32, tag=f"bsb{li}",
                                  name=f"bsb{li}")
                    nc.vector.tensor_copy(bsb[:], bps[:])
                with tc.tile_pool(name=f"lg{li}", bufs=2 * cfg.cpt + 2) as gp, \
                     tc.tile_pool(name=f"loh{li}", bufs=2 * cfg.cpt + 2) as ohp, \
                     tc.tile_pool(name=f"ls{li}", bufs=2) as sp, \
                     tc.tile_pool(name=f"lps{li}", bufs=2, space="PSUM") as pp:
                    for t in range(cfg.ntiles):
                        gts, ohs = [], []
                        erps = pp.tile([128, 64], f32, tag="erps", name="erps")
                        for c in range(cfg.cpt):
                            ch = t * cfg.cpt + c
                            gt = gp.tile([128, ROW], bf, tag="g", name="g")
                            nc.gpsimd.indirect_dma_start(
                                out=gt[:], out_offset=None, in_=ftab[:],
                                in_offset=bass.IndirectOffsetOnAxis(
                                    ap=idxsb[:, ch:ch + 1], axis=0))
                            gts.append(gt)
                            oh = ohp.tile([128, 128], bf, tag="oh", name="oh")
                            nc.vector.tensor_scalar(
                                out=oh[:], in0=iotaF[:],
                                scalar1=dpsb[:, ch:ch + 1], scalar2=None,
                                op0=ALU.is_equal)
                            ohs.append(oh)
                            ptp = pp.tile([128, 128], bf, tag="ptp", name="ptp")
                            nc.tensor.transpose(out=ptp[:], in_=oh[:],
                                                identity=identb[:])
                            oht = ohp.tile([128, 128], bf, tag="oht", name="oht",
                                           bufs=4)
                            nc.scalar.activation(oht[:], ptp[:], AF.Copy)
                            nc.tensor.matmul(out=erps[:, c * 4:(c + 1) * 4],
                                             lhsT=oht[:],
                                             rhs=ersb[:, t * 4:(t + 1) * 4],
                                             start=True, stop=True)
                        zz = sp.tile([128, 64], f32, tag="zz", name="zz")
                        for c in range(cfg.cpt):
                            nc.vector.tensor_add(zz[:, c * 4:(c + 1) * 4],
                                                 gts[c][:, Dout:Dout + 4],
                                                 erps[:, c * 4:(c + 1) * 4])
                        za = sp.tile([128, 64], f32, tag="za", name="za")
                        nc.vector.scalar_tensor_tensor(
                            out=za[:], in0=zz[:], scalar=NEG_SLOPE, in1=zz[:],
                            op0=ALU.mult, op1=ALU.max)
                        ee = sp.tile([128, 64], bf, tag="ee", name="ee")
                        nc.scalar.activation(ee[:], za[:], AF.Exp)
                        denps = pp.tile([128, 4], f32, tag="den", name="den")
                        rstps = [pp.tile([128, w], f32, tag=f"rst{j}",
                                         name=f"rst{j}", bufs=1)
                                 for j, (s, w) in enumerate(rcks)]
                        for c in range(cfg.cpt):
                            gt = gts[c]
                            for h in range(H):
                                nc.vector.scalar_tensor_tensor(
                                    out=gt[:, h * O:(h + 1) * O],
                                    in0=gt[:, h * O:(h + 1) * O], scalar=1.0,
                                    in1=ee[:, c * 4 + h:c * 4 + h + 1]
                                    .to_broadcast([128, O]),
                                    op0=ALU.mult, op1=ALU.mult)
                            nc.tensor.matmul(out=denps[:], lhsT=ohs[c][:],
                                             rhs=ee[:, c * 4:(c + 1) * 4],
                                             start=(c == 0),
                                             stop=(c == cfg.cpt - 1))
                            for j, (s, w) in enumerate(rcks):
                                nc.tensor.matmul(out=rstps[j][:],
                                                 lhsT=ohs[c][:],
                                                 rhs=gt[:, s:s + w],
                                                 start=(c == 0),
                                                 stop=(c == cfg.cpt - 1))
                        dcl = sp.tile([128, 4], f32, tag="dcl", name="dcl")
                        nc.vector.tensor_scalar_max(dcl[:], denps[:], 1e-9)
                        rec = sp.tile([128, 4], f32, tag="rec", name="rec")
                        nc.vector.reciprocal(rec[:], dcl[:])
                        y = sp.tile([128, Dout], f32, tag="y", name="y")
                        for h in range(H):
                            j = (h * O) // 512
                            s0 = (h * O) % 512
                            nc.vector.scalar_tensor_tensor(
                                out=y[:, h * O:(h + 1) * O],
                                in0=rstps[j][:, s0:s0 + O],
                                scalar=rec[:, h:h + 1],
                                in1=bsb[:, h * O:(h + 1) * O],
                                op0=ALU.mult, op1=ALU.add)
                        mn = sp.tile([128, Dout], f32, tag="mn", name="mn")
                        nc.vector.tensor_scalar_min(mn[:], y[:], 0.0)
                        ex = sp.tile([128, Dout], f32, tag="ex", name="ex")
                        nc.scalar.activation(ex[:], mn[:], AF.Exp)
                        y2 = sp.tile([128, Dout], f32, tag="y2", name="y2")
                        nc.vector.scalar_tensor_tensor(
                            out=y2[:], in0=y[:], scalar=0.0, in1=ex[:],
                            op0=ALU.max, op1=ALU.add)
                        xo = sp.tile([128, Dout], bf, tag="xo", name="xo")
                        nc.vector.tensor_scalar_add(xo[:], y2[:], -1.0)
                        # transpose xo -> next layer's xT (DRAM), via PE
                        Kn = Dout // 128
                        for kc in range(Kn):
                            ptp = pp.tile([128, 128], bf, tag="ptp",
                                          name="ptpx")
                            nc.tensor.transpose(
                                out=ptp[:], in_=xo[:, kc * 128:(kc + 1) * 128],
                                identity=identb[:])
                            xot = sp.tile([128, 128], bf, tag="xot", name="xot")
                            nc.scalar.activation(xot[:], ptp[:], AF.Copy)
                            nc.sync.dma_start(
                                xTd[li + 1][:, kc * cfg.nloc + t * 128:
                                            kc * cfg.nloc + (t + 1) * 128],
                                xot[:])
                        if li == 2:
                            nc.sync.dma_start(h3d[t * 128:(t + 1) * 128, :],
                                              xo[:])

            # ================= global attention pooling =================
            with tc.tile_pool(name="pool", bufs=1) as cp, \
                 tc.tile_pool(name="pools", bufs=3) as sp, \
                 tc.tile_pool(name="poolp", bufs=1, space="PSUM") as pp:
                h3Ts = cp.tile([128, 8 * cfg.nloc], bf, tag="h3Ts", name="h3Ts")
                nc.sync.dma_start(h3Ts[:], xTd[3][:])
                relu1 = cp.tile([128, cfg.nloc], bf, tag="relu1", name="relu1")
                nwin = (cfg.nloc + 511) // 512
                for w in range(nwin):
                    s = w * 512
                    ww = min(512, cfg.nloc - s)
                    ps = pp.tile([128, 512], f32, tag="g1", name="g1")
                    for kc in range(8):
                        nc.tensor.matmul(out=ps[:, :ww],
                                         lhsT=W["Wg1"][:, kc, :],
                                         rhs=h3Ts[:, kc * cfg.nloc + s:
                                                  kc * cfg.nloc + s + ww],
                                         start=(kc == 0), stop=(kc == 7))
                    nc.scalar.activation(relu1[:, s:s + ww], ps[:, :ww],
                                         AF.Relu, bias=bg1c[:])
                gps = pp.tile([128, 32], f32, tag="g2", name="g2")
                for t in range(cfg.ntiles):
                    nc.tensor.matmul(out=gps[:, t:t + 1],
                                     lhsT=relu1[:, t * 128:(t + 1) * 128],
                                     rhs=Wg2c[:], start=True, stop=True)
                eg = sp.tile([128, cfg.ntiles], bf, tag="eg", name="eg")
                nc.scalar.activation(eg[:], gps[:, :cfg.ntiles], AF.Exp,
                                     bias=bg2r[:])
                gd = pp.tile([cfg.gpad, 1], f32, tag="gd", name="gd")
                goha = sp.tile([128, cfg.ntiles * cfg.gpad], bf, tag="goha",
                               name="goha")
                for t in range(cfg.ntiles):
                    nc.tensor.matmul(out=gd[:],
                                     lhsT=GOHs[:, t * cfg.gpad:
                                               (t + 1) * cfg.gpad],
                                     rhs=eg[:, t:t + 1],
                                     start=(t == 0), stop=(t == cfg.ntiles - 1))
                    nc.vector.tensor_mul(
                        goha[:, t * cfg.gpad:(t + 1) * cfg.gpad],
                        GOHs[:, t * cfg.gpad:(t + 1) * cfg.gpad],
                        eg[:, t:t + 1].to_broadcast([128, cfg.gpad]))
                geps = [pp.tile([cfg.gpad, 512], f32, tag=f"ge{j}",
                                name=f"geps{j}") for j in range(2)]
                for t in range(cfg.ntiles):
                    h3t = sp.tile([128, D3], bf, tag="h3t", name="h3t")
                    nc.sync.dma_start(h3t[:], h3d[t * 128:(t + 1) * 128, :])
                    for j in range(2):
                        nc.tensor.matmul(
                            out=geps[j][:],
                            lhsT=goha[:, t * cfg.gpad:(t + 1) * cfg.gpad],
                            rhs=h3t[:, j * 512:(j + 1) * 512],
                            start=(t == 0), stop=(t == cfg.ntiles - 1))
                gdc = sp.tile([cfg.gpad, 1], f32, tag="gdc", name="gdc")
                nc.vector.tensor_scalar_max(gdc[:], gd[:], 1e-9)
                grc = sp.tile([cfg.gpad, 1], f32, tag="grc", name="grc")
                nc.vector.reciprocal(grc[:], gdc[:])
                ge = sp.tile([cfg.gpad, D3], f32, tag="ge", name="ge")
                for j in range(2):
                    nc.vector.tensor_scalar_mul(ge[:, j * 512:(j + 1) * 512],
                                                geps[j][:], grc[:, 0:1])
                geT = sp.tile([128, 8 * cfg.gpad], f32, tag="geT", name="geT")
                for kc in range(8):
                    pst = pp.tile([128, cfg.gpad], f32, tag="pst", name="pst")
                    nc.tensor.transpose(out=pst[:],
                                        in_=ge[:, kc * 128:(kc + 1) * 128],
                                        identity=ident32[:])
                    nc.vector.tensor_copy(
                        geT[:, kc * cfg.gpad:(kc + 1) * cfg.gpad], pst[:])
                for nm, bt, outp in [("Wmu", "bmu", mu_out),
                                     ("Wlv", "blv", lv_out)]:
                    mps = pp.tile([cfg.gpad, 128], f32, tag="mps", name="mps")
                    for kc in range(8):
                        nc.tensor.matmul(
                            out=mps[:],
                            lhsT=geT[:, kc * cfg.gpad:(kc + 1) * cfg.gpad],
                            rhs=W[nm][:, kc, :],
                            start=(kc == 0), stop=False)
                    nc.tensor.matmul(out=mps[:], lhsT=on32[:], rhs=W[bt][:],
                                     start=False, stop=True)
                    mo = sp.tile([cfg.gpad, 128], f32, tag="mo", name="mo")
                    nc.vector.tensor_copy(mo[:], mps[:])
                    nc.sync.dma_start(outp[:], mo[:])
    nc.compile()
    return nc


# ------------------------------------------------------ cached jitted runner
_BUILD_CACHE = {}
_RUN_CACHE = {}


def _get(key, fn):
    if key not in _BUILD_CACHE:
        _BUILD_CACHE[key] = fn()
    return _BUILD_CACHE[key]


def _make_runner(nc):
    import jax
    import numpy as _np
    from concourse import bass2jax, mybir
    from jax.sharding import Mesh, PartitionSpec
    from jax.experimental.shard_map import shard_map

    bass2jax.install_neuronx_cc_hook()
    n_cores = NCORES
    partition_name = (nc.partition_id_tensor.name
                      if nc.partition_id_tensor else None)
    in_names, out_names, out_avals, zero_outs = [], [], [], []
    for alloc in nc.m.functions[0].allocations:
        if not isinstance(alloc, mybir.MemoryLocationSet):
            continue
        name = alloc.memorylocations[0].name
        if alloc.kind == "ExternalInput":
            if name != partition_name:
                in_names.append(name)
        elif alloc.kind == "ExternalOutput":
            out_names.append(name)
            shape = tuple(alloc.tensor_shape)
            dtype = mybir.dt.np(alloc.dtype)
            out_avals.append(jax.core.ShapedArray(shape, dtype))
            zero_outs.append(_np.zeros(shape, dtype))
    n_params = len(in_names)
    n_outs = len(out_avals)
    all_names = list(in_names) + list(out_names)
    if partition_name is not None:
        all_names.append(partition_name)
    donate = tuple(range(n_params, n_params + n_outs))

    def _body(*args):
        operands = list(args)
        if partition_name is not None:
            operands.append(bass2jax.partition_id_tensor())
        outs = bass2jax._bass_exec_p.bind(
            *operands,
            out_avals=tuple(out_avals),
            in_names=tuple(all_names),
            out_names=tuple(out_names),
            lowering_input_output_aliases=(),
            sim_require_finite=True,
            sim_require_nnan=True,
            nc=nc,
        )
        return tuple(outs)

    mesh = Mesh(_np.asarray(jax.devices()[:n_cores]), ("core",))
    in_specs = (PartitionSpec("core"),) * (n_params + n_outs)
    out_specs = (PartitionSpec("core"),) * n_outs
    sharded = jax.jit(
        shard_map(_body, mesh=mesh, in_specs=in_specs, out_specs=out_specs,
                  check_rep=False),
        donate_argnums=donate, keep_unused=True)

    def run(in_maps):
        concat_in = [
            _np.concatenate([_np.asarray(in_maps[c][nm])
                             for c in range(n_cores)], axis=0)
            for nm in in_names]
        concat_zeros = [
            _np.zeros((n_cores * z.shape[0], *z.shape[1:]), z.dtype)
            for z in zero_outs]
        out_arrs = sharded(*concat_in, *concat_zeros)
        return [
            {nm: _np.asarray(out_arrs[i]).reshape(
                n_cores, *out_avals[i].shape)[c]
             for i, nm in enumerate(out_names)}
            for c in range(n_cores)]

    return run


def _run(nc, in_maps):
    if id(nc) not in _RUN_CACHE:
        _RUN_CACHE[id(nc)] = _make_runner(nc)
    return _RUN_CACHE[id(nc)](in_maps)


# ------------------------------------------------------------------ main entry
def kernel(node_feat, src, dst, graph_ids,
           W1, al1, ar1, b1, W2, al2, ar2, b2, W3, al3, ar3, b3,
           Wg1, bg1, Wg2, bg2, Wmu, bmu, Wlv, blv, cfg=None):
    cfg = cfg or CFG_FULL
    nc_ = cfg.ncores
    prep = host_prep(cfg, node_feat, src, dst, graph_ids)

    # pack weights
    wbmat = np.zeros((128, WBCOLS), BF16)
    wfmat = np.zeros((128, WFCOLS), np.float32)
    smb = np.zeros((1, SBCOLS), BF16)
    smf = np.zeros((1, SFCOLS), np.float32)

    def putb(nm, arr):
        o, ncols = SEG_B[nm]
        wbmat[:, o:o + ncols] = arr.reshape(128, ncols).astype(BF16)

    def putf(nm, arr):
        o, ncols = SEG_F[nm]
        wfmat[:, o:o + ncols] = arr.reshape(128, ncols).astype(np.float32)

    for li, (Wl, all_, arl) in enumerate([(W1, al1, ar1), (W2, al2, ar2),
                                          (W3, al3, ar3)]):
        Waug, Wr = fold_weights(np.asarray(Wl, np.float32),
                                np.asarray(all_, np.float32),
                                np.asarray(arl, np.float32))
        putb(f"W{li + 1}a", wstack(Waug))
        putb(f"Wr{li + 1}", wstack(Wr))
    putb("Wg1", wstack(np.asarray(Wg1, np.float32)))
    putf("Wmu", wstack(np.asarray(Wmu, np.float32)))
    putf("Wlv", wstack(np.asarray(Wlv, np.float32)))
    for nm, arr in [("b1", b1), ("b2", b2), ("b3", b3), ("Wg2", Wg2)]:
        o, ncols = SEG_SB[nm]
        smb[0, o:o + ncols] = np.asarray(arr, np.float32).reshape(-1).astype(BF16)
    for nm, arr in [("bg1", bg1), ("bmu", bmu), ("blv", blv), ("bg2", bg2)]:
        o, ncols = SEG_SF[nm]
        smf[0, o:o + ncols] = np.asarray(arr, np.float32).reshape(-1)

    SHARD = 128 // nc_
    ncF = _get(("FUSED",), lambda: build_fused(cfg))
    in_maps = [dict(
        x1T=prep["x1T"][c],
        idx=prep["idx32"][c],
        dpos=prep["dpos"][c],
        gid=prep["gid"][c],
        wb=np.ascontiguousarray(wbmat[c * SHARD:(c + 1) * SHARD]),
        wf=np.ascontiguousarray(wfmat[c * SHARD:(c + 1) * SHARD]),
        smb=smb, smf=smf) for c in range(nc_)]
    outs = _run(ncF, in_maps)
    mu = np.concatenate([outs[c]["mu"][:cfg.gpc] for c in range(nc_)], 0)
    lv = np.concatenate([outs[c]["lv"][:cfg.gpc] for c in range(nc_)], 0)
    return np.asarray(mu, np.float32), np.asarray(lv, np.float32)


# revision 3
# speedup vs baseline: 2.7220x; 1.2488x over previous
# kernel_fused.py — CrystalGCNEncoder (3-layer GAT + global attention pooling) on
# 8 trn2 NeuronCores, fused into ONE SPMD launch.
#
# vs the 7-launch baseline: the inter-layer feature-table exchange is an on-device
# AllGather (DRAM->Shared DRAM), the one-hot scatter/gather matrices are built on
# device from int32 slot indices (iota + is_equal), the big weight matrices are
# uploaded partition-sharded and AllGathered on device, and the inter-layer
# transposes run on the PE (identity matmul).  Host->device upload drops from
# ~1.2GB across 7 launches to ~11MB in one launch.
import numpy as np
import ml_dtypes

N, E, G = 20000, 320000, 200
F_IN, HID, H, LAT = 128, 128, 4, 128
O1, O2, O3 = HID // 2, HID, 2 * HID
D1, D2, D3 = H * O1, H * O2, H * O3          # 256, 512, 1024
NEG_SLOPE = 0.2
NCORES = 8
BF16 = ml_dtypes.bfloat16


def _row_elems(d):          # feat row: [d feats | 4 el | pad] bf16, 256B-multiple
    b = (d + 4) * 2
    return ((b + 255) // 256 * 256) // 2


class Cfg:
    def __init__(self, n, e, g, ntiles, cpt, ncores=NCORES):
        self.n, self.e, self.g, self.ncores = n, e, g, ncores
        self.gpc = g // ncores
        self.ntiles = ntiles
        self.nloc = ntiles * 128
        self.nstar = self.nloc * ncores
        self.cpt = cpt
        self.tpe = cpt * 128
        self.eloc = ntiles * self.tpe
        self.nch = self.eloc // 128
        self.gpad = 32


CFG_FULL = Cfg(N, E, G, ntiles=21, cpt=16)

LAYERS_DIMS = [(F_IN, D1), (D1, D2), (D2, D3)]

# ---------------------------------------------------------------- weight packing
# wbmat: [128, WBCOLS] bf16, partition-sharded 16 rows/core, AllGathered on device
# wfmat: [128, WFCOLS] f32, same
# smalls_b: [1, SB] bf16 replicated;  smalls_f: [1, SF] f32 replicated
def _seg_layout():
    wb, wf, sb, sf = {}, {}, {}, {}
    ob = of = osb = osf = 0

    def addb(name, cols):
        nonlocal ob
        wb[name] = (ob, cols)
        ob += cols

    def addf(name, cols):
        nonlocal of
        wf[name] = (of, cols)
        of += cols

    def addsb(name, cols):
        nonlocal osb
        sb[name] = (osb, cols)
        osb += cols

    def addsf(name, cols):
        nonlocal osf
        sf[name] = (osf, cols)
        osf += cols

    addb("W1a", 1 * (D1 + 4)); addb("W2a", 2 * (D2 + 4)); addb("W3a", 4 * (D3 + 4))
    addb("Wr1", 1 * 4); addb("Wr2", 2 * 4); addb("Wr3", 4 * 4)
    addb("Wg1", 8 * 128)
    addf("Wmu", 8 * 128); addf("Wlv", 8 * 128)
    addsb("b1", D1); addsb("b2", D2); addsb("b3", D3); addsb("Wg2", 128)
    addsf("bg1", 128); addsf("bmu", 128); addsf("blv", 128); addsf("bg2", 1)
    return wb, ob, wf, of, sb, osb, sf, osf


SEG_B, WBCOLS, SEG_F, WFCOLS, SEG_SB, SBCOLS, SEG_SF, SFCOLS = _seg_layout()


def fold_weights(W, al, ar):
    Din, D = W.shape
    Hh, O = al.shape
    Wl = np.einsum("iho,ho->ih", W.reshape(Din, Hh, O), al)
    Wr = np.einsum("iho,ho->ih", W.reshape(Din, Hh, O), ar)
    return np.concatenate([W, Wl], 1).astype(np.float32), Wr.astype(np.float32)


def wstack(Waug):
    """[Din, C] -> [128, Din/128, C] (partition-major K chunks)."""
    Din, C = Waug.shape
    return np.ascontiguousarray(Waug.reshape(Din // 128, 128, C).transpose(1, 0, 2))


def _colchunks(c):
    out, s = [], 0
    while s < c:
        w = min(512, c - s)
        out.append((s, w))
        s += w
    return out


# ------------------------------------------------------------------ host prep
def host_prep(cfg, node_feat, src, dst, graph_ids):
    n, nc_ = cfg.n, cfg.ncores
    node_feat = np.asarray(node_feat, np.float32)
    src = np.asarray(src).astype(np.int64)
    dst = np.asarray(dst).astype(np.int64)
    graph_ids = np.asarray(graph_ids).astype(np.int64)

    gbounds = np.arange(nc_ + 1) * cfg.gpc
    nbounds = np.searchsorted(graph_ids, gbounds)
    core_of_node = np.searchsorted(nbounds, np.arange(n), side="right") - 1
    indeg = np.bincount(dst, minlength=n)

    glob2slot = np.zeros(n, np.int64)
    tile_of_node = np.zeros(n, np.int64)
    slotpos_of_node = np.zeros(n, np.int64)
    for c in range(nc_):
        nodes = np.arange(nbounds[c], nbounds[c + 1])
        assert len(nodes) <= cfg.nloc
        order = nodes[np.argsort(-indeg[nodes], kind="stable")]
        loads = np.zeros(cfg.ntiles, np.int64)
        counts = np.zeros(cfg.ntiles, np.int64)
        for nd in order:
            free = np.nonzero(counts < 128)[0]
            tgt = free[np.argmin(loads[free])]
            tile_of_node[nd] = tgt
            slotpos_of_node[nd] = counts[tgt]
            glob2slot[nd] = c * cfg.nloc + tgt * 128 + counts[tgt]
            counts[tgt] += 1
            loads[tgt] += indeg[nd]
        assert loads.max() <= cfg.tpe

    edge_core = core_of_node[dst]
    idx32_l, dpos_l, gid_l = [], [], []
    for c in range(nc_):
        eids = np.nonzero(edge_core == c)[0]
        assert len(eids) <= cfg.eloc
        src_slot = np.zeros(cfg.eloc, np.int64)
        dst_pos = np.full(cfg.eloc, -1, np.int64)
        et = tile_of_node[dst[eids]]
        for t in range(cfg.ntiles):
            sel = eids[et == t]
            assert len(sel) <= cfg.tpe
            b = t * cfg.tpe
            src_slot[b : b + len(sel)] = glob2slot[src[sel]]
            dst_pos[b : b + len(sel)] = slotpos_of_node[dst[sel]]
        # per-chunk indices [128, nch] (edge i of chunk ch at [i, ch])
        idx32_l.append(np.ascontiguousarray(
            src_slot.reshape(cfg.nch, 128).T).astype(np.int32))
        dpos_l.append(np.ascontiguousarray(
            dst_pos.reshape(cfg.nch, 128).T).astype(np.float32))
        # local graph id per slot [128, ntiles] (-1 for pad slots)
        gid = np.full((cfg.ntiles, 128), -1, np.int64)
        nodes = np.arange(nbounds[c], nbounds[c + 1])
        gid[tile_of_node[nodes], slotpos_of_node[nodes]] = (
            graph_ids[nodes] - c * cfg.gpc)
        gid_l.append(np.ascontiguousarray(gid.T).astype(np.float32))

    x1 = np.zeros((cfg.nstar, F_IN), np.float32)
    x1[glob2slot] = node_feat
    # per-core transposed feature block [128, nloc]
    x1T_l = [np.ascontiguousarray(
        x1[c * cfg.nloc:(c + 1) * cfg.nloc].T).astype(BF16)
        for c in range(nc_)]
    return dict(idx32=idx32_l, dpos=dpos_l, gid=gid_l, x1T=x1T_l)


# ------------------------------------------------------------------ the kernel
def build_fused(cfg):
    import concourse.bass as bass
    import concourse.tile as tile
    from concourse import bacc, mybir

    bf = mybir.dt.bfloat16
    f32 = mybir.dt.float32
    i32 = mybir.dt.int32
    AF = mybir.ActivationFunctionType
    ALU = mybir.AluOpType
    RG = [list(range(cfg.ncores))]
    SHARD = 128 // cfg.ncores

    nc = bacc.Bacc("TRN2", target_bir_lowering=False, debug=False,
                   num_devices=cfg.ncores)
    x1T_in = nc.dram_tensor("x1T", [128, cfg.nloc], bf, kind="ExternalInput").ap()
    idx_in = nc.dram_tensor("idx", [128, cfg.nch], i32, kind="ExternalInput").ap()
    dp_in = nc.dram_tensor("dpos", [128, cfg.nch], f32, kind="ExternalInput").ap()
    gid_in = nc.dram_tensor("gid", [128, cfg.ntiles], f32,
                            kind="ExternalInput").ap()
    wb_in = nc.dram_tensor("wb", [SHARD, WBCOLS], bf, kind="ExternalInput").ap()
    wf_in = nc.dram_tensor("wf", [SHARD, WFCOLS], f32, kind="ExternalInput").ap()
    sb_in = nc.dram_tensor("smb", [1, SBCOLS], bf, kind="ExternalInput").ap()
    sf_in = nc.dram_tensor("smf", [1, SFCOLS], f32, kind="ExternalInput").ap()
    mu_out = nc.dram_tensor("mu", [cfg.gpad, 128], f32, kind="ExternalOutput").ap()
    lv_out = nc.dram_tensor("lv", [cfg.gpad, 128], f32, kind="ExternalOutput").ap()

    with tile.TileContext(nc) as tc:
        with tc.tile_pool(name="dram", bufs=1, space="DRAM") as dpool, \
             tc.tile_pool(name="glob", bufs=1) as gl:
            # ---- distribute weights: partition-sharded upload + AllGather
            wbb = dpool.tile([SHARD, WBCOLS], bf, tag="wbb", name="wbb")
            wfb = dpool.tile([SHARD, WFCOLS], f32, tag="wfb", name="wfb")
            nc.sync.dma_start(wbb[:], wb_in[:])
            nc.sync.dma_start(wfb[:], wf_in[:])
            wbfull = dpool.tile([128, WBCOLS], bf, tag="wbfull", name="wbfull",
                                addr_space="Shared")
            wffull = dpool.tile([128, WFCOLS], f32, tag="wffull", name="wffull",
                                addr_space="Shared")
            nc.gpsimd.collective_compute(
                "AllGather", ALU.bypass, replica_groups=RG,
                ins=[wbb.opt()], outs=[wbfull.opt()])
            nc.gpsimd.collective_compute(
                "AllGather", ALU.bypass, replica_groups=RG,
                ins=[wfb.opt()], outs=[wffull.opt()])

            W = {}
            for nm, K, C in [("W1a", 1, D1 + 4), ("W2a", 2, D2 + 4),
                             ("W3a", 4, D3 + 4), ("Wr1", 1, 4), ("Wr2", 2, 4),
                             ("Wr3", 4, 4), ("Wg1", 8, 128)]:
                o, ncols = SEG_B[nm]
                t = gl.tile([128, K, C], bf, tag=nm, name=nm)
                nc.sync.dma_start(t[:], wbfull[:, o:o + ncols])
                W[nm] = t
            for nm in ["Wmu", "Wlv"]:
                o, ncols = SEG_F[nm]
                t = gl.tile([128, 8, 128], f32, tag=nm, name=nm)
                nc.sync.dma_start(t[:], wffull[:, o:o + ncols])
                W[nm] = t
            for nm, dt_ in [("b1", bf), ("b2", bf), ("b3", bf)]:
                o, ncols = SEG_SB[nm]
                t = gl.tile([1, ncols], dt_, tag=nm, name=nm)
                nc.sync.dma_start(t[:], sb_in[0:1, o:o + ncols])
                W[nm] = t
            o, ncols = SEG_SB["Wg2"]
            Wg2c = gl.tile([128, 1], bf, tag="Wg2", name="Wg2c")
            nc.sync.dma_start(Wg2c[:],
                              sb_in[0:1, o:o + 128].rearrange("a b -> b a"))
            bg1c = gl.tile([128, 1], f32, tag="bg1", name="bg1c")
            o, _ = SEG_SF["bg1"]
            nc.sync.dma_start(bg1c[:],
                              sf_in[0:1, o:o + 128].rearrange("a b -> b a"))
            for nm in ["bmu", "blv"]:
                o, ncols = SEG_SF[nm]
                t = gl.tile([1, 128], f32, tag=nm, name=nm)
                nc.sync.dma_start(t[:], sf_in[0:1, o:o + ncols])
                W[nm] = t
            bg2r = gl.tile([128, 1], f32, tag="bg2", name="bg2r")
            o, _ = SEG_SF["bg2"]
            nc.sync.dma_start(bg2r[:],
                              sf_in[0:1, o:o + 1].to_broadcast([128, 1]))

            # ---- constants
            iotaF = gl.tile([128, 128], f32, tag="iotaF", name="iotaF")
            nc.gpsimd.iota(iotaF[:], pattern=[[1, 128]], base=0,
                           channel_multiplier=0,
                           allow_small_or_imprecise_dtypes=True)
            iotaC = gl.tile([128, 1], f32, tag="iotaC", name="iotaC")
            nc.gpsimd.iota(iotaC[:], pattern=[[0, 1]], base=0,
                           channel_multiplier=1,
                           allow_small_or_imprecise_dtypes=True)
            identb = gl.tile([128, 128], bf, tag="identb", name="identb")
            nc.vector.tensor_scalar(out=identb[:], in0=iotaF[:],
                                    scalar1=iotaC[:, 0:1], scalar2=None,
                                    op0=ALU.is_equal)
            iota32 = gl.tile([128, 32], f32, tag="iota32", name="iota32")
            nc.gpsimd.iota(iota32[:], pattern=[[1, 32]], base=0,
                           channel_multiplier=0,
                           allow_small_or_imprecise_dtypes=True)
            ident32 = gl.tile([32, 32], f32, tag="ident32", name="ident32")
            nc.vector.tensor_scalar(out=ident32[:], in0=iota32[0:32, :],
                                    scalar1=iotaC[0:32, 0:1], scalar2=None,
                                    op0=ALU.is_equal)
            onesr = gl.tile([1, 128], bf, tag="onesr", name="onesr")
            nc.vector.memset(onesr[:], 1.0)
            on32 = gl.tile([1, 32], f32, tag="on32", name="on32")
            nc.vector.memset(on32[:], 1.0)

            idxsb = gl.tile([128, cfg.nch], i32, tag="idxsb", name="idxsb")
            nc.sync.dma_start(idxsb[:], idx_in[:])
            dpsb = gl.tile([128, cfg.nch], f32, tag="dpsb", name="dpsb")
            nc.sync.dma_start(dpsb[:], dp_in[:])
            gidsb = gl.tile([128, cfg.ntiles], f32, tag="gidsb", name="gidsb")
            nc.sync.dma_start(gidsb[:], gid_in[:])
            GOHs = gl.tile([128, cfg.ntiles * cfg.gpad], bf, tag="GOHs",
                           name="GOHs")
            for t in range(cfg.ntiles):
                nc.vector.tensor_scalar(
                    out=GOHs[:, t * cfg.gpad:(t + 1) * cfg.gpad],
                    in0=iota32[:], scalar1=gidsb[:, t:t + 1], scalar2=None,
                    op0=ALU.is_equal)
            ersb = gl.tile([128, cfg.ntiles * 4], bf, tag="ersb", name="ersb")

            # DRAM intermediates
            ftabs, fwr = [], []
            for li, (Din, Dout) in enumerate(LAYERS_DIMS):
                ROW = _row_elems(Dout)
                fwr.append(dpool.tile([cfg.nloc, ROW], bf, tag=f"fw{li}",
                                      name=f"fw{li}"))
                ftabs.append(dpool.tile([cfg.nstar, ROW], bf, tag=f"ft{li}",
                                        name=f"ft{li}", addr_space="Shared"))
            xTd = [None]
            for li, (Din, Dout) in enumerate(LAYERS_DIMS[1:] + [(D3, 0)]):
                xTd.append(dpool.tile([128, (Din // 128) * cfg.nloc], bf,
                                      tag=f"xT{li + 1}", name=f"xT{li + 1}"))
            h3d = dpool.tile([cfg.nloc, D3], bf, tag="h3d", name="h3d")

            # ================= three GAT layers =================
            for li, (Din, Dout) in enumerate(LAYERS_DIMS):
                K = Din // 128
                ROW = _row_elems(Dout)
                O = Dout // H
                Wa, Wr = W[f"W{li + 1}a"], W[f"Wr{li + 1}"]
                brow = W[f"b{li + 1}"]
                cks = _colchunks(Dout + 4)
                rcks = _colchunks(Dout)
                xsrc = x1T_in if li == 0 else xTd[li]

                # ---------- P: feat = x @ [W|W@al], er = x @ (W@ar)
                with tc.tile_pool(name=f"px{li}", bufs=3) as xp, \
                     tc.tile_pool(name=f"pp{li}", bufs=2, space="PSUM") as pp, \
                     tc.tile_pool(name=f"po{li}", bufs=3) as op:
                    for t in range(cfg.ntiles):
                        pa = [pp.tile([128, w], f32, tag=f"pa{j}", name=f"pa{j}")
                              for j, (s, w) in enumerate(cks)]
                        pe = pp.tile([128, 4], f32, tag="pe", name="pe")
                        for kc in range(K):
                            xt = xp.tile([128, 128], bf, tag="xt", name="xt")
                            nc.sync.dma_start(
                                xt[:], xsrc[:, kc * cfg.nloc + t * 128:
                                            kc * cfg.nloc + (t + 1) * 128])
                            for j, (s, w) in enumerate(cks):
                                nc.tensor.matmul(out=pa[j][:], lhsT=xt[:],
                                                 rhs=Wa[:, kc, s:s + w],
                                                 start=(kc == 0),
                                                 stop=(kc == K - 1))
                            nc.tensor.matmul(out=pe[:], lhsT=xt[:],
                                             rhs=Wr[:, kc, :],
                                             start=(kc == 0), stop=(kc == K - 1))
                        ft = op.tile([128, ROW], bf, tag="ft", name="ft")
                        for j, (s, w) in enumerate(cks):
                            nc.vector.tensor_copy(ft[:, s:s + w], pa[j][:])
                        nc.vector.tensor_copy(ersb[:, t * 4:(t + 1) * 4], pe[:])
                        nc.sync.dma_start(fwr[li][t * 128:(t + 1) * 128, :ROW],
                                          ft[:])

                # ---------- exchange feature tables
                nc.gpsimd.collective_compute(
                    "AllGather", ALU.bypass, replica_groups=RG,
                    ins=[fwr[li].opt()], outs=[ftabs[li].opt()])
                ftab = ftabs[li]

                # ---------- L: gather, edge softmax, aggregate, ELU
                with tc.tile_pool(name=f"lb{li}", bufs=1,
                                  space="PSUM") as bpp:
                    bps = bpp.tile([128, Dout], f32, tag="bias", name="bps")
                    for (s, w) in rcks:
                        nc.tensor.matmul(out=bps[:, s:s + w], lhsT=onesr[:],
                                         rhs=brow[:, s:s + w],
                                         start=True, stop=True)
                    bsb = gl.tile([128, Dout], f32, tag=f"bsb{li}",
                                  name=f"bsb{li}")
                    nc.vector.tensor_copy(bsb[:], bps[:])
                with tc.tile_pool(name=f"lg{li}", bufs=2 * cfg.cpt + 2) as gp, \
                     tc.tile_pool(name=f"loh{li}", bufs=2 * cfg.cpt + 2) as ohp, \
                     tc.tile_pool(name=f"ls{li}", bufs=2) as sp, \
                     tc.tile_pool(name=f"lps{li}", bufs=2, space="PSUM") as pp:
                    for t in range(cfg.ntiles):
                        gts, ohs = [], []
                        erps = pp.tile([128, 64], f32, tag="erps", name="erps")
                        for c in range(cfg.cpt):
                            ch = t * cfg.cpt + c
                            gt = gp.tile([128, ROW], bf, tag="g", name="g")
                            nc.gpsimd.indirect_dma_start(
                                out=gt[:], out_offset=None, in_=ftab[:],
                                in_offset=bass.IndirectOffsetOnAxis(
                                    ap=idxsb[:, ch:ch + 1], axis=0))
                            gts.append(gt)
                            oh = ohp.tile([128, 128], bf, tag="oh", name="oh")
                            nc.vector.tensor_scalar(
                                out=oh[:], in0=iotaF[:],
                                scalar1=dpsb[:, ch:ch + 1], scalar2=None,
                                op0=ALU.is_equal)
                            ohs.append(oh)
                            ptp = pp.tile([128, 128], bf, tag="ptp", name="ptp")
                            nc.tensor.transpose(out=ptp[:], in_=oh[:],
                                                identity=identb[:])
                            oht = ohp.tile([128, 128], bf, tag="oht", name="oht",
                                           bufs=4)
                            nc.scalar.activation(oht[:], ptp[:], AF.Copy)
                            nc.tensor.matmul(out=erps[:, c * 4:(c + 1) * 4],
                                             lhsT=oht[:],
                                             rhs=ersb[:, t * 4:(t + 1) * 4],
                                             start=True, stop=True)
                        zz = sp.tile([128, 64], f32, tag="zz", name="zz")
                        for c in range(cfg.cpt):
                            nc.vector.tensor_add(zz[:, c * 4:(c + 1) * 4],
                                                 gts[c][:, Dout:Dout + 4],
                                                 erps[:, c * 4:(c + 1) * 4])
                        za = sp.tile([128, 64], f32, tag="za", name="za")
                        nc.vector.scalar_tensor_tensor(
                            out=za[:], in0=zz[:], scalar=NEG_SLOPE, in1=zz[:],
                            op0=ALU.mult, op1=ALU.max)
                        ee = sp.tile([128, 64], bf, tag="ee", name="ee")
                        nc.scalar.activation(ee[:], za[:], AF.Exp)
                        denps = pp.tile([128, 4], f32, tag="den", name="den")
                        rstps = [pp.tile([128, w], f32, tag=f"rst{j}",
                                         name=f"rst{j}", bufs=1)
                                 for j, (s, w) in enumerate(rcks)]
                        for c in range(cfg.cpt):
                            gt = gts[c]
                            for h in range(H):
                                nc.vector.scalar_tensor_tensor(
                                    out=gt[:, h * O:(h + 1) * O],
                                    in0=gt[:, h * O:(h + 1) * O], scalar=1.0,
                                    in1=ee[:, c * 4 + h:c * 4 + h + 1]
                                    .to_broadcast([128, O]),
                                    op0=ALU.mult, op1=ALU.mult)
                            nc.tensor.matmul(out=denps[:], lhsT=ohs[c][:],
                                             rhs=ee[:, c * 4:(c + 1) * 4],
                                             start=(c == 0),
                                             stop=(c == cfg.cpt - 1))
                            for j, (s, w) in enumerate(rcks):
                                nc.tensor.matmul(out=rstps[j][:],
                                                 lhsT=ohs[c][:],
                                                 rhs=gt[:, s:s + w],
                                                 start=(c == 0),
                                                 stop=(c == cfg.cpt - 1))
                        dcl = sp.tile([128, 4], f32, tag="dcl", name="dcl")
                        nc.vector.tensor_scalar_max(dcl[:], denps[:], 1e-9)
                        rec = sp.tile([128, 4], f32, tag="rec", name="rec")
                        nc.vector.reciprocal(rec[:], dcl[:])
                        y = sp.tile([128, Dout], f32, tag="y", name="y")
                        for h in range(H):
                            j = (h * O) // 512
                            s0 = (h * O) % 512
                            nc.vector.scalar_tensor_tensor(
                                out=y[:, h * O:(h + 1) * O],
                                in0=rstps[j][:, s0:s0 + O],
                                scalar=rec[:, h:h + 1],
                                in1=bsb[:, h * O:(h + 1) * O],
                                op0=ALU.mult, op1=ALU.add)
                        mn = sp.tile([128, Dout], f32, tag="mn", name="mn")
                        nc.vector.tensor_scalar_min(mn[:], y[:], 0.0)
                        ex = sp.tile([128, Dout], f32, tag="ex", name="ex")
                        nc.scalar.activation(ex[:], mn[:], AF.Exp)
                        y2 = sp.tile([128, Dout], f32, tag="y2", name="y2")
                        nc.vector.scalar_tensor_tensor(
                            out=y2[:], in0=y[:], scalar=0.0, in1=ex[:],
                            op0=ALU.max, op1=ALU.add)
                        xo = sp.tile([128, Dout], bf, tag="xo", name="xo")
                        nc.vector.tensor_scalar_add(xo[:], y2[:], -1.0)
                        # transpose xo -> next layer's xT (DRAM), via PE
                        Kn = Dout // 128
                        for kc in range(Kn):
                            ptp = pp.tile([128, 128], bf, tag="ptp",
                                          name="ptpx")
                            nc.tensor.transpose(
                                out=ptp[:], in_=xo[:, kc * 128:(kc + 1) * 128],
                                identity=identb[:])
                            xot = sp.tile([128, 128], bf, tag="xot", name="xot")
                            nc.scalar.activation(xot[:], ptp[:], AF.Copy)
                            nc.sync.dma_start(
                                xTd[li + 1][:, kc * cfg.nloc + t * 128:
                                            kc * cfg.nloc + (t + 1) * 128],
                                xot[:])
                        if li == 2:
                            nc.sync.dma_start(h3d[t * 128:(t + 1) * 128, :],
                                              xo[:])

            # ================= global attention pooling =================
            with tc.tile_pool(name="pool", bufs=1) as cp, \
                 tc.tile_pool(name="pools", bufs=3) as sp, \
                 tc.tile_pool(name="poolp", bufs=1, space="PSUM") as pp:
                h3Ts = cp.tile([128, 8 * cfg.nloc], bf, tag="h3Ts", name="h3Ts")
                nc.sync.dma_start(h3Ts[:], xTd[3][:])
                relu1 = cp.tile([128, cfg.nloc], bf, tag="relu1", name="relu1")
                nwin = (cfg.nloc + 511) // 512
                for w in range(nwin):
                    s = w * 512
                    ww = min(512, cfg.nloc - s)
                    ps = pp.tile([128, 512], f32, tag="g1", name="g1")
                    for kc in range(8):
                        nc.tensor.matmul(out=ps[:, :ww],
                                         lhsT=W["Wg1"][:, kc, :],
                                         rhs=h3Ts[:, kc * cfg.nloc + s:
                                                  kc * cfg.nloc + s + ww],
                                         start=(kc == 0), stop=(kc == 7))
                    nc.scalar.activation(relu1[:, s:s + ww], ps[:, :ww],
                                         AF.Relu, bias=bg1c[:])
                gps = pp.tile([128, 32], f32, tag="g2", name="g2")
                for t in range(cfg.ntiles):
                    nc.tensor.matmul(out=gps[:, t:t + 1],
                                     lhsT=relu1[:, t * 128:(t + 1) * 128],
                                     rhs=Wg2c[:], start=True, stop=True)
                eg = sp.tile([128, cfg.ntiles], bf, tag="eg", name="eg")
                nc.scalar.activation(eg[:], gps[:, :cfg.ntiles], AF.Exp,
                                     bias=bg2r[:])
                gd = pp.tile([cfg.gpad, 1], f32, tag="gd", name="gd")
                goha = sp.tile([128, cfg.ntiles * cfg.gpad], bf, tag="goha",
                               name="goha")
                for t in range(cfg.ntiles):
                    nc.tensor.matmul(out=gd[:],
                                     lhsT=GOHs[:, t * cfg.gpad:
                                               (t + 1) * cfg.gpad],
                                     rhs=eg[:, t:t + 1],
                                     start=(t == 0), stop=(t == cfg.ntiles - 1))
                    nc.vector.tensor_mul(
                        goha[:, t * cfg.gpad:(t + 1) * cfg.gpad],
                        GOHs[:, t * cfg.gpad:(t + 1) * cfg.gpad],
                        eg[:, t:t + 1].to_broadcast([128, cfg.gpad]))
                geps = [pp.tile([cfg.gpad, 512], f32, tag=f"ge{j}",
                                name=f"geps{j}") for j in range(2)]
                for t in range(cfg.ntiles):
                    h3t = sp.tile([128, D3], bf, tag="h3t", name="h3t")
                    nc.sync.dma_start(h3t[:], h3d[t * 128:(t + 1) * 128, :])
                    for j in range(2):
                        nc.tensor.matmul(
                            out=geps[j][:],
                            lhsT=goha[:, t * cfg.gpad:(t + 1) * cfg.gpad],
                            rhs=h3t[:, j * 512:(j + 1) * 512],
                            start=(t == 0), stop=(t == cfg.ntiles - 1))
                gdc = sp.tile([cfg.gpad, 1], f32, tag="gdc", name="gdc")
                nc.vector.tensor_scalar_max(gdc[:], gd[:], 1e-9)
                grc = sp.tile([cfg.gpad, 1], f32, tag="grc", name="grc")
                nc.vector.reciprocal(grc[:], gdc[:])
                ge = sp.tile([cfg.gpad, D3], f32, tag="ge", name="ge")
                for j in range(2):
                    nc.vector.tensor_scalar_mul(ge[:, j * 512:(j + 1) * 512],
                                                geps[j][:], grc[:, 0:1])
                geT = sp.tile([128, 8 * cfg.gpad], f32, tag="geT", name="geT")
                for kc in range(8):
                    pst = pp.tile([128, cfg.gpad], f32, tag="pst", name="pst")
                    nc.tensor.transpose(out=pst[:],
                                        in_=ge[:, kc * 128:(kc + 1) * 128],
                                        identity=ident32[:])
                    nc.vector.tensor_copy(
                        geT[:, kc * cfg.gpad:(kc + 1) * cfg.gpad], pst[:])
                for nm, bt, outp in [("Wmu", "bmu", mu_out),
                                     ("Wlv", "blv", lv_out)]:
                    mps = pp.tile([cfg.gpad, 128], f32, tag="mps", name="mps")
                    for kc in range(8):
                        nc.tensor.matmul(
                            out=mps[:],
                            lhsT=geT[:, kc * cfg.gpad:(kc + 1) * cfg.gpad],
                            rhs=W[nm][:, kc, :],
                            start=(kc == 0), stop=False)
                    nc.tensor.matmul(out=mps[:], lhsT=on32[:], rhs=W[bt][:],
                                     start=False, stop=True)
                    mo = sp.tile([cfg.gpad, 128], f32, tag="mo", name="mo")
                    nc.vector.tensor_copy(mo[:], mps[:])
                    nc.sync.dma_start(outp[:], mo[:])
    nc.compile()
    return nc


# ------------------------------------------------------ cached jitted runner
_BUILD_CACHE = {}
_RUN_CACHE = {}


def _get(key, fn):
    if key not in _BUILD_CACHE:
        _BUILD_CACHE[key] = fn()
    return _BUILD_CACHE[key]


def _make_runner(nc):
    import jax
    import numpy as _np
    from concourse import bass2jax, mybir
    from jax.sharding import Mesh, PartitionSpec
    from jax.experimental.shard_map import shard_map

    bass2jax.install_neuronx_cc_hook()
    n_cores = NCORES
    partition_name = (nc.partition_id_tensor.name
                      if nc.partition_id_tensor else None)
    in_names, out_names, out_avals, zero_outs = [], [], [], []
    for alloc in nc.m.functions[0].allocations:
        if not isinstance(alloc, mybir.MemoryLocationSet):
            continue
        name = alloc.memorylocations[0].name
        if alloc.kind == "ExternalInput":
            if name != partition_name:
                in_names.append(name)
        elif alloc.kind == "ExternalOutput":
            out_names.append(name)
            shape = tuple(alloc.tensor_shape)
            dtype = mybir.dt.np(alloc.dtype)
            out_avals.append(jax.core.ShapedArray(shape, dtype))
            zero_outs.append(_np.zeros(shape, dtype))
    n_params = len(in_names)
    n_outs = len(out_avals)
    all_names = list(in_names) + list(out_names)
    if partition_name is not None:
        all_names.append(partition_name)
    donate = tuple(range(n_params, n_params + n_outs))

    def _body(*args):
        operands = list(args)
        if partition_name is not None:
            operands.append(bass2jax.partition_id_tensor())
        outs = bass2jax._bass_exec_p.bind(
            *operands,
            out_avals=tuple(out_avals),
            in_names=tuple(all_names),
            out_names=tuple(out_names),
            lowering_input_output_aliases=(),
            sim_require_finite=True,
            sim_require_nnan=True,
            nc=nc,
        )
        return tuple(outs)

    mesh = Mesh(_np.asarray(jax.devices()[:n_cores]), ("core",))
    in_specs = (PartitionSpec("core"),) * (n_params + n_outs)
    out_specs = (PartitionSpec("core"),) * n_outs
    sharded = jax.jit(
        shard_map(_body, mesh=mesh, in_specs=in_specs, out_specs=out_specs,
                  check_rep=False),
        donate_argnums=donate, keep_unused=True)

    sharding = jax.sharding.NamedSharding(mesh, PartitionSpec("core"))

    def prefetch(in_maps):
        concat_in = [
            _np.concatenate([_np.asarray(in_maps[c][nm])
                             for c in range(n_cores)], axis=0)
            for nm in in_names]
        return [jax.device_put(a, sharding) for a in concat_in]

    def run(args):
        if isinstance(args, list) and args and isinstance(args[0], dict):
            args = prefetch(args)
        concat_zeros = [
            _np.zeros((n_cores * z.shape[0], *z.shape[1:]), z.dtype)
            for z in zero_outs]
        out_arrs = sharded(*args, *concat_zeros)
        return [
            {nm: _np.asarray(out_arrs[i]).reshape(
                n_cores, *out_avals[i].shape)[c]
             for i, nm in enumerate(out_names)}
            for c in range(n_cores)]

    run.prefetch = prefetch
    return run


def _run(nc, in_maps):
    if id(nc) not in _RUN_CACHE:
        _RUN_CACHE[id(nc)] = _make_runner(nc)
    return _RUN_CACHE[id(nc)](in_maps)


def _prefetch(nc, in_maps):
    if id(nc) not in _RUN_CACHE:
        _RUN_CACHE[id(nc)] = _make_runner(nc)
    pf = _RUN_CACHE[id(nc)].prefetch(in_maps)
    for a in pf:
        a.block_until_ready()
    return pf


# ------------------------------------------------------------------ main entry
def kernel(node_feat, src, dst, graph_ids,
           W1, al1, ar1, b1, W2, al2, ar2, b2, W3, al3, ar3, b3,
           Wg1, bg1, Wg2, bg2, Wmu, bmu, Wlv, blv, cfg=None):
    cfg = cfg or CFG_FULL
    nc_ = cfg.ncores
    prep = host_prep(cfg, node_feat, src, dst, graph_ids)

    # pack weights
    wbmat = np.zeros((128, WBCOLS), BF16)
    wfmat = np.zeros((128, WFCOLS), np.float32)
    smb = np.zeros((1, SBCOLS), BF16)
    smf = np.zeros((1, SFCOLS), np.float32)

    def putb(nm, arr):
        o, ncols = SEG_B[nm]
        wbmat[:, o:o + ncols] = arr.reshape(128, ncols).astype(BF16)

    def putf(nm, arr):
        o, ncols = SEG_F[nm]
        wfmat[:, o:o + ncols] = arr.reshape(128, ncols).astype(np.float32)

    for li, (Wl, all_, arl) in enumerate([(W1, al1, ar1), (W2, al2, ar2),
                                          (W3, al3, ar3)]):
        Waug, Wr = fold_weights(np.asarray(Wl, np.float32),
                                np.asarray(all_, np.float32),
                                np.asarray(arl, np.float32))
        putb(f"W{li + 1}a", wstack(Waug))
        putb(f"Wr{li + 1}", wstack(Wr))
    putb("Wg1", wstack(np.asarray(Wg1, np.float32)))
    putf("Wmu", wstack(np.asarray(Wmu, np.float32)))
    putf("Wlv", wstack(np.asarray(Wlv, np.float32)))
    for nm, arr in [("b1", b1), ("b2", b2), ("b3", b3), ("Wg2", Wg2)]:
        o, ncols = SEG_SB[nm]
        smb[0, o:o + ncols] = np.asarray(arr, np.float32).reshape(-1).astype(BF16)
    for nm, arr in [("bg1", bg1), ("bmu", bmu), ("blv", blv), ("bg2", bg2)]:
        o, ncols = SEG_SF[nm]
        smf[0, o:o + ncols] = np.asarray(arr, np.float32).reshape(-1)

    SHARD = 128 // nc_
    ncF = _get(("FUSED",), lambda: build_fused(cfg))
    in_maps = [dict(
        x1T=prep["x1T"][c],
        idx=prep["idx32"][c],
        dpos=prep["dpos"][c],
        gid=prep["gid"][c],
        wb=np.ascontiguousarray(wbmat[c * SHARD:(c + 1) * SHARD]),
        wf=np.ascontiguousarray(wfmat[c * SHARD:(c + 1) * SHARD]),
        smb=smb, smf=smf) for c in range(nc_)]
    handle = _prefetch(ncF, in_maps)
    outs = _run(ncF, handle)
    mu = np.concatenate([outs[c]["mu"][:cfg.gpc] for c in range(nc_)], 0)
    lv = np.concatenate([outs[c]["lv"][:cfg.gpc] for c in range(nc_)], 0)
    return np.asarray(mu, np.float32), np.asarray(lv, np.float32)


# revision 4
# speedup vs baseline: 4.1637x; 1.5297x over previous
# kernel_fused.py — CrystalGCNEncoder (3-layer GAT + global attention pooling) on
# 8 trn2 NeuronCores, fused into ONE SPMD launch.
#
# vs the 7-launch baseline: the inter-layer feature-table exchange is an on-device
# AllGather (DRAM->Shared DRAM), the one-hot scatter/gather matrices are built on
# device from int32 slot indices (iota + is_equal), the big weight matrices are
# uploaded partition-sharded and AllGathered on device, and the inter-layer
# transposes run on the PE (identity matmul).  Host->device upload drops from
# ~1.2GB across 7 launches to ~11MB in one launch.
import numpy as np
import ml_dtypes

N, E, G = 20000, 320000, 200
F_IN, HID, H, LAT = 128, 128, 4, 128
O1, O2, O3 = HID // 2, HID, 2 * HID
D1, D2, D3 = H * O1, H * O2, H * O3          # 256, 512, 1024
NEG_SLOPE = 0.2
NCORES = 8
BF16 = ml_dtypes.bfloat16


def _row_elems(d):          # feat row: [d feats | 4 el | pad] bf16, 256B-multiple
    b = (d + 4) * 2
    return ((b + 255) // 256 * 256) // 2


class Cfg:
    def __init__(self, n, e, g, ntiles, cpt, ncores=NCORES):
        self.n, self.e, self.g, self.ncores = n, e, g, ncores
        self.gpc = g // ncores
        self.ntiles = ntiles
        self.nloc = ntiles * 128
        self.nstar = self.nloc * ncores
        self.cpt = cpt
        self.tpe = cpt * 128
        self.eloc = ntiles * self.tpe
        self.nch = self.eloc // 128
        self.gpad = 32


CFG_FULL = Cfg(N, E, G, ntiles=21, cpt=16)

LAYERS_DIMS = [(F_IN, D1), (D1, D2), (D2, D3)]

# ---------------------------------------------------------------- weight packing
# wbmat: [128, WBCOLS] bf16, partition-sharded 16 rows/core, AllGathered on device
# wfmat: [128, WFCOLS] f32, same
# smalls_b: [1, SB] bf16 replicated;  smalls_f: [1, SF] f32 replicated
def _seg_layout():
    wb, wf, sb, sf = {}, {}, {}, {}
    ob = of = osb = osf = 0

    def addb(name, cols):
        nonlocal ob
        wb[name] = (ob, cols)
        ob += cols

    def addf(name, cols):
        nonlocal of
        wf[name] = (of, cols)
        of += cols

    def addsb(name, cols):
        nonlocal osb
        sb[name] = (osb, cols)
        osb += cols

    def addsf(name, cols):
        nonlocal osf
        sf[name] = (osf, cols)
        osf += cols

    addb("W1a", 1 * (D1 + 4)); addb("W2a", 2 * (D2 + 4)); addb("W3a", 4 * (D3 + 4))
    addb("Wr1", 1 * 4); addb("Wr2", 2 * 4); addb("Wr3", 4 * 4)
    addb("Wg1", 8 * 128)
    addf("Wmu", 8 * 128); addf("Wlv", 8 * 128)
    addsb("b1", D1); addsb("b2", D2); addsb("b3", D3); addsb("Wg2", 128)
    addsf("bg1", 128); addsf("bmu", 128); addsf("blv", 128); addsf("bg2", 1)
    return wb, ob, wf, of, sb, osb, sf, osf


SEG_B, WBCOLS, SEG_F, WFCOLS, SEG_SB, SBCOLS, SEG_SF, SFCOLS = _seg_layout()


def fold_weights(W, al, ar):
    Din, D = W.shape
    Hh, O = al.shape
    Wl = np.einsum("iho,ho->ih", W.reshape(Din, Hh, O), al)
    Wr = np.einsum("iho,ho->ih", W.reshape(Din, Hh, O), ar)
    return np.concatenate([W, Wl], 1).astype(np.float32), Wr.astype(np.float32)


def wstack(Waug):
    """[Din, C] -> [128, Din/128, C] (partition-major K chunks)."""
    Din, C = Waug.shape
    return np.ascontiguousarray(Waug.reshape(Din // 128, 128, C).transpose(1, 0, 2))


def _colchunks(c):
    out, s = [], 0
    while s < c:
        w = min(512, c - s)
        out.append((s, w))
        s += w
    return out


# ------------------------------------------------------------------ host prep
def host_prep(cfg, node_feat, src, dst, graph_ids):
    n, nc_ = cfg.n, cfg.ncores
    node_feat = np.asarray(node_feat, np.float32)
    src = np.asarray(src).astype(np.int64)
    dst = np.asarray(dst).astype(np.int64)
    graph_ids = np.asarray(graph_ids).astype(np.int64)

    gbounds = np.arange(nc_ + 1) * cfg.gpc
    nbounds = np.searchsorted(graph_ids, gbounds)
    core_of_node = np.searchsorted(nbounds, np.arange(n), side="right") - 1
    indeg = np.bincount(dst, minlength=n)

    glob2slot = np.zeros(n, np.int64)
    tile_of_node = np.zeros(n, np.int64)
    slotpos_of_node = np.zeros(n, np.int64)
    for c in range(nc_):
        nodes = np.arange(nbounds[c], nbounds[c + 1])
        assert len(nodes) <= cfg.nloc
        order = nodes[np.argsort(-indeg[nodes], kind="stable")]
        loads = np.zeros(cfg.ntiles, np.int64)
        counts = np.zeros(cfg.ntiles, np.int64)
        for nd in order:
            free = np.nonzero(counts < 128)[0]
            tgt = free[np.argmin(loads[free])]
            tile_of_node[nd] = tgt
            slotpos_of_node[nd] = counts[tgt]
            glob2slot[nd] = c * cfg.nloc + tgt * 128 + counts[tgt]
            counts[tgt] += 1
            loads[tgt] += indeg[nd]
        assert loads.max() <= cfg.tpe

    edge_core = core_of_node[dst]
    idx32_l, dpos_l, gid_l = [], [], []
    for c in range(nc_):
        eids = np.nonzero(edge_core == c)[0]
        assert len(eids) <= cfg.eloc
        src_slot = np.zeros(cfg.eloc, np.int64)
        dst_pos = np.full(cfg.eloc, -1, np.int64)
        et = tile_of_node[dst[eids]]
        for t in range(cfg.ntiles):
            sel = eids[et == t]
            assert len(sel) <= cfg.tpe
            b = t * cfg.tpe
            src_slot[b : b + len(sel)] = glob2slot[src[sel]]
            dst_pos[b : b + len(sel)] = slotpos_of_node[dst[sel]]
        # per-chunk indices [128, nch] (edge i of chunk ch at [i, ch])
        idx32_l.append(np.ascontiguousarray(
            src_slot.reshape(cfg.nch, 128).T).astype(np.uint16))
        dpos_l.append(np.ascontiguousarray(
            dst_pos.reshape(cfg.nch, 128).T).astype(np.int8))
        # local graph id per slot [128, ntiles] (-1 for pad slots)
        gid = np.full((cfg.ntiles, 128), -1, np.int64)
        nodes = np.arange(nbounds[c], nbounds[c + 1])
        gid[tile_of_node[nodes], slotpos_of_node[nodes]] = (
            graph_ids[nodes] - c * cfg.gpc)
        gid_l.append(np.ascontiguousarray(gid.T).astype(np.int8))

    x1 = np.zeros((cfg.nstar, F_IN), np.float32)
    x1[glob2slot] = node_feat
    # per-core transposed feature block [128, nloc]
    x1T_l = [np.ascontiguousarray(
        x1[c * cfg.nloc:(c + 1) * cfg.nloc].T).astype(BF16)
        for c in range(nc_)]
    return dict(idx32=idx32_l, dpos=dpos_l, gid=gid_l, x1T=x1T_l)


# ------------------------------------------------------------------ the kernel
def build_fused(cfg):
    import concourse.bass as bass
    import concourse.tile as tile
    from concourse import bacc, mybir

    bf = mybir.dt.bfloat16
    f32 = mybir.dt.float32
    i32 = mybir.dt.int32
    AF = mybir.ActivationFunctionType
    ALU = mybir.AluOpType
    RG = [list(range(cfg.ncores))]
    SHARD = 128 // cfg.ncores

    nc = bacc.Bacc("TRN2", target_bir_lowering=False, debug=False,
                   num_devices=cfg.ncores)
    u16 = mybir.dt.uint16
    i8 = mybir.dt.int8
    x1T_in = nc.dram_tensor("x1T", [128, cfg.nloc], bf, kind="ExternalInput").ap()
    idx_in = nc.dram_tensor("idx", [128, cfg.nch], u16, kind="ExternalInput").ap()
    dp_in = nc.dram_tensor("dpos", [128, cfg.nch], i8, kind="ExternalInput").ap()
    gid_in = nc.dram_tensor("gid", [128, cfg.ntiles], i8,
                            kind="ExternalInput").ap()
    wb_in = nc.dram_tensor("wb", [SHARD, WBCOLS], bf, kind="ExternalInput").ap()
    wf_in = nc.dram_tensor("wf", [SHARD, WFCOLS], f32, kind="ExternalInput").ap()
    sb_in = nc.dram_tensor("smb", [1, SBCOLS], bf, kind="ExternalInput").ap()
    sf_in = nc.dram_tensor("smf", [1, SFCOLS], f32, kind="ExternalInput").ap()
    mlv_out = nc.dram_tensor("mlv", [2 * cfg.gpad, 128], f32,
                             kind="ExternalOutput").ap()
    mu_out = mlv_out[0:cfg.gpad, :]
    lv_out = mlv_out[cfg.gpad:2 * cfg.gpad, :]

    with tile.TileContext(nc) as tc:
        with tc.tile_pool(name="dram", bufs=1, space="DRAM") as dpool, \
             tc.tile_pool(name="glob", bufs=1) as gl:
            # ---- distribute weights: partition-sharded upload + AllGather
            wbb = dpool.tile([SHARD, WBCOLS], bf, tag="wbb", name="wbb")
            wfb = dpool.tile([SHARD, WFCOLS], f32, tag="wfb", name="wfb")
            nc.sync.dma_start(wbb[:], wb_in[:])
            nc.sync.dma_start(wfb[:], wf_in[:])
            wbfull = dpool.tile([128, WBCOLS], bf, tag="wbfull", name="wbfull",
                                addr_space="Shared")
            wffull = dpool.tile([128, WFCOLS], f32, tag="wffull", name="wffull",
                                addr_space="Shared")
            nc.gpsimd.collective_compute(
                "AllGather", ALU.bypass, replica_groups=RG,
                ins=[wbb.opt()], outs=[wbfull.opt()])
            nc.gpsimd.collective_compute(
                "AllGather", ALU.bypass, replica_groups=RG,
                ins=[wfb.opt()], outs=[wffull.opt()])

            W = {}
            for nm, K, C in [("W1a", 1, D1 + 4), ("W2a", 2, D2 + 4),
                             ("W3a", 4, D3 + 4), ("Wr1", 1, 4), ("Wr2", 2, 4),
                             ("Wr3", 4, 4), ("Wg1", 8, 128)]:
                o, ncols = SEG_B[nm]
                t = gl.tile([128, K, C], bf, tag=nm, name=nm)
                nc.sync.dma_start(t[:], wbfull[:, o:o + ncols])
                W[nm] = t
            for nm in ["Wmu", "Wlv"]:
                o, ncols = SEG_F[nm]
                t = gl.tile([128, 8, 128], f32, tag=nm, name=nm)
                nc.sync.dma_start(t[:], wffull[:, o:o + ncols])
                W[nm] = t
            for nm, dt_ in [("b1", bf), ("b2", bf), ("b3", bf)]:
                o, ncols = SEG_SB[nm]
                t = gl.tile([1, ncols], dt_, tag=nm, name=nm)
                nc.sync.dma_start(t[:], sb_in[0:1, o:o + ncols])
                W[nm] = t
            o, ncols = SEG_SB["Wg2"]
            Wg2c = gl.tile([128, 1], bf, tag="Wg2", name="Wg2c")
            nc.sync.dma_start(Wg2c[:],
                              sb_in[0:1, o:o + 128].rearrange("a b -> b a"))
            bg1c = gl.tile([128, 1], f32, tag="bg1", name="bg1c")
            o, _ = SEG_SF["bg1"]
            nc.sync.dma_start(bg1c[:],
                              sf_in[0:1, o:o + 128].rearrange("a b -> b a"))
            for nm in ["bmu", "blv"]:
                o, ncols = SEG_SF[nm]
                t = gl.tile([1, 128], f32, tag=nm, name=nm)
                nc.sync.dma_start(t[:], sf_in[0:1, o:o + ncols])
                W[nm] = t
            bg2r = gl.tile([128, 1], f32, tag="bg2", name="bg2r")
            o, _ = SEG_SF["bg2"]
            nc.sync.dma_start(bg2r[:],
                              sf_in[0:1, o:o + 1].to_broadcast([128, 1]))

            # ---- constants
            iotaF = gl.tile([128, 128], f32, tag="iotaF", name="iotaF")
            nc.gpsimd.iota(iotaF[:], pattern=[[1, 128]], base=0,
                           channel_multiplier=0,
                           allow_small_or_imprecise_dtypes=True)
            iotaC = gl.tile([128, 1], f32, tag="iotaC", name="iotaC")
            nc.gpsimd.iota(iotaC[:], pattern=[[0, 1]], base=0,
                           channel_multiplier=1,
                           allow_small_or_imprecise_dtypes=True)
            identb = gl.tile([128, 128], bf, tag="identb", name="identb")
            nc.vector.tensor_scalar(out=identb[:], in0=iotaF[:],
                                    scalar1=iotaC[:, 0:1], scalar2=None,
                                    op0=ALU.is_equal)
            iota32 = gl.tile([128, 32], f32, tag="iota32", name="iota32")
            nc.gpsimd.iota(iota32[:], pattern=[[1, 32]], base=0,
                           channel_multiplier=0,
                           allow_small_or_imprecise_dtypes=True)
            ident32 = gl.tile([32, 32], f32, tag="ident32", name="ident32")
            nc.vector.tensor_scalar(out=ident32[:], in0=iota32[0:32, :],
                                    scalar1=iotaC[0:32, 0:1], scalar2=None,
                                    op0=ALU.is_equal)
            onesr = gl.tile([1, 128], bf, tag="onesr", name="onesr")
            nc.vector.memset(onesr[:], 1.0)
            on32 = gl.tile([1, 32], f32, tag="on32", name="on32")
            nc.vector.memset(on32[:], 1.0)

            idxu = gl.tile([128, cfg.nch], u16, tag="idxu", name="idxu")
            nc.sync.dma_start(idxu[:], idx_in[:])
            idxsb = gl.tile([128, cfg.nch], i32, tag="idxsb", name="idxsb")
            nc.vector.tensor_copy(idxsb[:], idxu[:])
            dpu = gl.tile([128, cfg.nch], i8, tag="dpu", name="dpu")
            nc.sync.dma_start(dpu[:], dp_in[:])
            dpsb = gl.tile([128, cfg.nch], f32, tag="dpsb", name="dpsb")
            nc.vector.tensor_copy(dpsb[:], dpu[:])
            gidu = gl.tile([128, cfg.ntiles], i8, tag="gidu", name="gidu")
            nc.sync.dma_start(gidu[:], gid_in[:])
            gidsb = gl.tile([128, cfg.ntiles], f32, tag="gidsb", name="gidsb")
            nc.vector.tensor_copy(gidsb[:], gidu[:])
            GOHs = gl.tile([128, cfg.ntiles * cfg.gpad], bf, tag="GOHs",
                           name="GOHs")
            for t in range(cfg.ntiles):
                nc.vector.tensor_scalar(
                    out=GOHs[:, t * cfg.gpad:(t + 1) * cfg.gpad],
                    in0=iota32[:], scalar1=gidsb[:, t:t + 1], scalar2=None,
                    op0=ALU.is_equal)
            ersb = gl.tile([128, cfg.ntiles * 4], bf, tag="ersb", name="ersb")

            # DRAM intermediates
            ftabs, fwr = [], []
            for li, (Din, Dout) in enumerate(LAYERS_DIMS):
                ROW = _row_elems(Dout)
                fwr.append(dpool.tile([cfg.nloc, ROW], bf, tag=f"fw{li}",
                                      name=f"fw{li}"))
                ftabs.append(dpool.tile([cfg.nstar, ROW], bf, tag=f"ft{li}",
                                        name=f"ft{li}", addr_space="Shared"))
            xTd = [None]
            for li, (Din, Dout) in enumerate(LAYERS_DIMS[1:] + [(D3, 0)]):
                xTd.append(dpool.tile([128, (Din // 128) * cfg.nloc], bf,
                                      tag=f"xT{li + 1}", name=f"xT{li + 1}"))
            h3d = dpool.tile([cfg.nloc, D3], bf, tag="h3d", name="h3d")

            # ================= three GAT layers =================
            for li, (Din, Dout) in enumerate(LAYERS_DIMS):
                K = Din // 128
                ROW = _row_elems(Dout)
                O = Dout // H
                Wa, Wr = W[f"W{li + 1}a"], W[f"Wr{li + 1}"]
                brow = W[f"b{li + 1}"]
                cks = _colchunks(Dout + 4)
                rcks = _colchunks(Dout)
                xsrc = x1T_in if li == 0 else xTd[li]

                # ---------- P: feat = x @ [W|W@al], er = x @ (W@ar)
                with tc.tile_pool(name=f"px{li}", bufs=3) as xp, \
                     tc.tile_pool(name=f"pp{li}", bufs=2, space="PSUM") as pp, \
                     tc.tile_pool(name=f"po{li}", bufs=3) as op:
                    for t in range(cfg.ntiles):
                        pa = [pp.tile([128, w], f32, tag=f"pa{j}", name=f"pa{j}")
                              for j, (s, w) in enumerate(cks)]
                        pe = pp.tile([128, 4], f32, tag="pe", name="pe")
                        for kc in range(K):
                            xt = xp.tile([128, 128], bf, tag="xt", name="xt")
                            nc.sync.dma_start(
                                xt[:], xsrc[:, kc * cfg.nloc + t * 128:
                                            kc * cfg.nloc + (t + 1) * 128])
                            for j, (s, w) in enumerate(cks):
                                nc.tensor.matmul(out=pa[j][:], lhsT=xt[:],
                                                 rhs=Wa[:, kc, s:s + w],
                                                 start=(kc == 0),
                                                 stop=(kc == K - 1))
                            nc.tensor.matmul(out=pe[:], lhsT=xt[:],
                                             rhs=Wr[:, kc, :],
                                             start=(kc == 0), stop=(kc == K - 1))
                        ft = op.tile([128, ROW], bf, tag="ft", name="ft")
                        for j, (s, w) in enumerate(cks):
                            nc.vector.tensor_copy(ft[:, s:s + w], pa[j][:])
                        nc.vector.tensor_copy(ersb[:, t * 4:(t + 1) * 4], pe[:])
                        nc.sync.dma_start(fwr[li][t * 128:(t + 1) * 128, :ROW],
                                          ft[:])

                # ---------- exchange feature tables
                nc.gpsimd.collective_compute(
                    "AllGather", ALU.bypass, replica_groups=RG,
                    ins=[fwr[li].opt()], outs=[ftabs[li].opt()])
                ftab = ftabs[li]

                # ---------- L: gather, edge softmax, aggregate, ELU
                with tc.tile_pool(name=f"lb{li}", bufs=1,
                                  space="PSUM") as bpp:
                    bps = bpp.tile([128, Dout], f32, tag="bias", name="bps")
                    for (s, w) in rcks:
                        nc.tensor.matmul(out=bps[:, s:s + w], lhsT=onesr[:],
                                         rhs=brow[:, s:s + w],
                                         start=True, stop=True)
                    bsb = gl.tile([128, Dout], f32, tag=f"bsb{li}",
                                  name=f"bsb{li}")
                    nc.vector.tensor_copy(bsb[:], bps[:])
                with tc.tile_pool(name=f"lg{li}", bufs=2 * cfg.cpt + 2) as gp, \
                     tc.tile_pool(name=f"loh{li}", bufs=2 * cfg.cpt + 2) as ohp, \
                     tc.tile_pool(name=f"ls{li}", bufs=2) as sp, \
                     tc.tile_pool(name=f"lps{li}", bufs=2, space="PSUM") as pp:
                    for t in range(cfg.ntiles):
                        gts, ohs = [], []
                        erps = pp.tile([128, 64], f32, tag="erps", name="erps")
                        for c in range(cfg.cpt):
                            ch = t * cfg.cpt + c
                            gt = gp.tile([128, ROW], bf, tag="g", name="g")
                            nc.gpsimd.indirect_dma_start(
                                out=gt[:], out_offset=None, in_=ftab[:],
                                in_offset=bass.IndirectOffsetOnAxis(
                                    ap=idxsb[:, ch:ch + 1], axis=0))
                            gts.append(gt)
                            oh = ohp.tile([128, 128], bf, tag="oh", name="oh")
                            nc.vector.tensor_scalar(
                                out=oh[:], in0=iotaF[:],
                                scalar1=dpsb[:, ch:ch + 1], scalar2=None,
                                op0=ALU.is_equal)
                            ohs.append(oh)
                            ptp = pp.tile([128, 128], bf, tag="ptp", name="ptp")
                            nc.tensor.transpose(out=ptp[:], in_=oh[:],
                                                identity=identb[:])
                            oht = ohp.tile([128, 128], bf, tag="oht", name="oht",
                                           bufs=4)
                            nc.scalar.activation(oht[:], ptp[:], AF.Copy)
                            nc.tensor.matmul(out=erps[:, c * 4:(c + 1) * 4],
                                             lhsT=oht[:],
                                             rhs=ersb[:, t * 4:(t + 1) * 4],
                                             start=True, stop=True)
                        zz = sp.tile([128, 64], f32, tag="zz", name="zz")
                        for c in range(cfg.cpt):
                            nc.vector.tensor_add(zz[:, c * 4:(c + 1) * 4],
                                                 gts[c][:, Dout:Dout + 4],
                                                 erps[:, c * 4:(c + 1) * 4])
                        za = sp.tile([128, 64], f32, tag="za", name="za")
                        nc.vector.scalar_tensor_tensor(
                            out=za[:], in0=zz[:], scalar=NEG_SLOPE, in1=zz[:],
                            op0=ALU.mult, op1=ALU.max)
                        ee = sp.tile([128, 64], bf, tag="ee", name="ee")
                        nc.scalar.activation(ee[:], za[:], AF.Exp)
                        denps = pp.tile([128, 4], f32, tag="den", name="den")
                        rstps = [pp.tile([128, w], f32, tag=f"rst{j}",
                                         name=f"rst{j}", bufs=1)
                                 for j, (s, w) in enumerate(rcks)]
                        for c in range(cfg.cpt):
                            gt = gts[c]
                            for h in range(H):
                                nc.vector.scalar_tensor_tensor(
                                    out=gt[:, h * O:(h + 1) * O],
                                    in0=gt[:, h * O:(h + 1) * O], scalar=1.0,
                                    in1=ee[:, c * 4 + h:c * 4 + h + 1]
                                    .to_broadcast([128, O]),
                                    op0=ALU.mult, op1=ALU.mult)
                            nc.tensor.matmul(out=denps[:], lhsT=ohs[c][:],
                                             rhs=ee[:, c * 4:(c + 1) * 4],
                                             start=(c == 0),
                                             stop=(c == cfg.cpt - 1))
                            for j, (s, w) in enumerate(rcks):
                                nc.tensor.matmul(out=rstps[j][:],
                                                 lhsT=ohs[c][:],
                                                 rhs=gt[:, s:s + w],
                                                 start=(c == 0),
                                                 stop=(c == cfg.cpt - 1))
                        dcl = sp.tile([128, 4], f32, tag="dcl", name="dcl")
                        nc.vector.tensor_scalar_max(dcl[:], denps[:], 1e-9)
                        rec = sp.tile([128, 4], f32, tag="rec", name="rec")
                        nc.vector.reciprocal(rec[:], dcl[:])
                        y = sp.tile([128, Dout], f32, tag="y", name="y")
                        for h in range(H):
                            j = (h * O) // 512
                            s0 = (h * O) % 512
                            nc.vector.scalar_tensor_tensor(
                                out=y[:, h * O:(h + 1) * O],
                                in0=rstps[j][:, s0:s0 + O],
                                scalar=rec[:, h:h + 1],
                                in1=bsb[:, h * O:(h + 1) * O],
                                op0=ALU.mult, op1=ALU.add)
                        mn = sp.tile([128, Dout], f32, tag="mn", name="mn")
                        nc.vector.tensor_scalar_min(mn[:], y[:], 0.0)
                        ex = sp.tile([128, Dout], f32, tag="ex", name="ex")
                        nc.scalar.activation(ex[:], mn[:], AF.Exp)
                        y2 = sp.tile([128, Dout], f32, tag="y2", name="y2")
                        nc.vector.scalar_tensor_tensor(
                            out=y2[:], in0=y[:], scalar=0.0, in1=ex[:],
                            op0=ALU.max, op1=ALU.add)
                        xo = sp.tile([128, Dout], bf, tag="xo", name="xo")
                        nc.vector.tensor_scalar_add(xo[:], y2[:], -1.0)
                        # transpose xo -> next layer's xT (DRAM), via PE
                        Kn = Dout // 128
                        for kc in range(Kn):
                            ptp = pp.tile([128, 128], bf, tag="ptp",
                                          name="ptpx")
                            nc.tensor.transpose(
                                out=ptp[:], in_=xo[:, kc * 128:(kc + 1) * 128],
                                identity=identb[:])
                            xot = sp.tile([128, 128], bf, tag="xot", name="xot")
                            nc.scalar.activation(xot[:], ptp[:], AF.Copy)
                            nc.sync.dma_start(
                                xTd[li + 1][:, kc * cfg.nloc + t * 128:
                                            kc * cfg.nloc + (t + 1) * 128],
                                xot[:])
                        if li == 2:
                            nc.sync.dma_start(h3d[t * 128:(t + 1) * 128, :],
                                              xo[:])

            # ================= global attention pooling =================
            with tc.tile_pool(name="pool", bufs=1) as cp, \
                 tc.tile_pool(name="pools", bufs=3) as sp, \
                 tc.tile_pool(name="poolp", bufs=1, space="PSUM") as pp:
                h3Ts = cp.tile([128, 8 * cfg.nloc], bf, tag="h3Ts", name="h3Ts")
                nc.sync.dma_start(h3Ts[:], xTd[3][:])
                relu1 = cp.tile([128, cfg.nloc], bf, tag="relu1", name="relu1")
                nwin = (cfg.nloc + 511) // 512
                for w in range(nwin):
                    s = w * 512
                    ww = min(512, cfg.nloc - s)
                    ps = pp.tile([128, 512], f32, tag="g1", name="g1")
                    for kc in range(8):
                        nc.tensor.matmul(out=ps[:, :ww],
                                         lhsT=W["Wg1"][:, kc, :],
                                         rhs=h3Ts[:, kc * cfg.nloc + s:
                                                  kc * cfg.nloc + s + ww],
                                         start=(kc == 0), stop=(kc == 7))
                    nc.scalar.activation(relu1[:, s:s + ww], ps[:, :ww],
                                         AF.Relu, bias=bg1c[:])
                gps = pp.tile([128, 32], f32, tag="g2", name="g2")
                for t in range(cfg.ntiles):
                    nc.tensor.matmul(out=gps[:, t:t + 1],
                                     lhsT=relu1[:, t * 128:(t + 1) * 128],
                                     rhs=Wg2c[:], start=True, stop=True)
                eg = sp.tile([128, cfg.ntiles], bf, tag="eg", name="eg")
                nc.scalar.activation(eg[:], gps[:, :cfg.ntiles], AF.Exp,
                                     bias=bg2r[:])
                gd = pp.tile([cfg.gpad, 1], f32, tag="gd", name="gd")
                goha = sp.tile([128, cfg.ntiles * cfg.gpad], bf, tag="goha",
                               name="goha")
                for t in range(cfg.ntiles):
                    nc.tensor.matmul(out=gd[:],
                                     lhsT=GOHs[:, t * cfg.gpad:
                                               (t + 1) * cfg.gpad],
                                     rhs=eg[:, t:t + 1],
                                     start=(t == 0), stop=(t == cfg.ntiles - 1))
                    nc.vector.tensor_mul(
                        goha[:, t * cfg.gpad:(t + 1) * cfg.gpad],
                        GOHs[:, t * cfg.gpad:(t + 1) * cfg.gpad],
                        eg[:, t:t + 1].to_broadcast([128, cfg.gpad]))
                geps = [pp.tile([cfg.gpad, 512], f32, tag=f"ge{j}",
                                name=f"geps{j}") for j in range(2)]
                for t in range(cfg.ntiles):
                    h3t = sp.tile([128, D3], bf, tag="h3t", name="h3t")
                    nc.sync.dma_start(h3t[:], h3d[t * 128:(t + 1) * 128, :])
                    for j in range(2):
                        nc.tensor.matmul(
                            out=geps[j][:],
                            lhsT=goha[:, t * cfg.gpad:(t + 1) * cfg.gpad],
                            rhs=h3t[:, j * 512:(j + 1) * 512],
                            start=(t == 0), stop=(t == cfg.ntiles - 1))
                gdc = sp.tile([cfg.gpad, 1], f32, tag="gdc", name="gdc")
                nc.vector.tensor_scalar_max(gdc[:], gd[:], 1e-9)
                grc = sp.tile([cfg.gpad, 1], f32, tag="grc", name="grc")
                nc.vector.reciprocal(grc[:], gdc[:])
                ge = sp.tile([cfg.gpad, D3], f32, tag="ge", name="ge")
                for j in range(2):
                    nc.vector.tensor_scalar_mul(ge[:, j * 512:(j + 1) * 512],
                                                geps[j][:], grc[:, 0:1])
                geT = sp.tile([128, 8 * cfg.gpad], f32, tag="geT", name="geT")
                for kc in range(8):
                    pst = pp.tile([128, cfg.gpad], f32, tag="pst", name="pst")
                    nc.tensor.transpose(out=pst[:],
                                        in_=ge[:, kc * 128:(kc + 1) * 128],
                                        identity=ident32[:])
                    nc.vector.tensor_copy(
                        geT[:, kc * cfg.gpad:(kc + 1) * cfg.gpad], pst[:])
                for nm, bt, outp in [("Wmu", "bmu", mu_out),
                                     ("Wlv", "blv", lv_out)]:
                    mps = pp.tile([cfg.gpad, 128], f32, tag="mps", name="mps")
                    for kc in range(8):
                        nc.tensor.matmul(
                            out=mps[:],
                            lhsT=geT[:, kc * cfg.gpad:(kc + 1) * cfg.gpad],
                            rhs=W[nm][:, kc, :],
                            start=(kc == 0), stop=False)
                    nc.tensor.matmul(out=mps[:], lhsT=on32[:], rhs=W[bt][:],
                                     start=False, stop=True)
                    mo = sp.tile([cfg.gpad, 128], f32, tag="mo", name="mo")
                    nc.vector.tensor_copy(mo[:], mps[:])
                    nc.sync.dma_start(outp[:], mo[:])
    nc.compile()
    return nc


# ------------------------------------------------------ cached jitted runner
_BUILD_CACHE = {}
_RUN_CACHE = {}


def _get(key, fn):
    if key not in _BUILD_CACHE:
        _BUILD_CACHE[key] = fn()
    return _BUILD_CACHE[key]


def _make_runner(nc):
    import jax
    import numpy as _np
    from concourse import bass2jax, mybir
    from jax.sharding import Mesh, PartitionSpec
    from jax.experimental.shard_map import shard_map

    bass2jax.install_neuronx_cc_hook()
    n_cores = NCORES
    partition_name = (nc.partition_id_tensor.name
                      if nc.partition_id_tensor else None)
    in_names, out_names, out_avals, zero_outs = [], [], [], []
    for alloc in nc.m.functions[0].allocations:
        if not isinstance(alloc, mybir.MemoryLocationSet):
            continue
        name = alloc.memorylocations[0].name
        if alloc.kind == "ExternalInput":
            if name != partition_name:
                in_names.append(name)
        elif alloc.kind == "ExternalOutput":
            out_names.append(name)
            shape = tuple(alloc.tensor_shape)
            dtype = mybir.dt.np(alloc.dtype)
            out_avals.append(jax.core.ShapedArray(shape, dtype))
            zero_outs.append(_np.zeros(shape, dtype))
    n_params = len(in_names)
    n_outs = len(out_avals)
    all_names = list(in_names) + list(out_names)
    if partition_name is not None:
        all_names.append(partition_name)
    donate = tuple(range(n_params, n_params + n_outs))

    def _body(*args):
        operands = list(args)
        if partition_name is not None:
            operands.append(bass2jax.partition_id_tensor())
        outs = bass2jax._bass_exec_p.bind(
            *operands,
            out_avals=tuple(out_avals),
            in_names=tuple(all_names),
            out_names=tuple(out_names),
            lowering_input_output_aliases=(),
            sim_require_finite=True,
            sim_require_nnan=True,
            nc=nc,
        )
        return tuple(outs)

    mesh = Mesh(_np.asarray(jax.devices()[:n_cores]), ("core",))
    in_specs = (PartitionSpec("core"),) * (n_params + n_outs)
    out_specs = (PartitionSpec("core"),) * n_outs
    sharded = jax.jit(
        shard_map(_body, mesh=mesh, in_specs=in_specs, out_specs=out_specs,
                  check_rep=False),
        donate_argnums=donate, keep_unused=True)

    sharding = jax.sharding.NamedSharding(mesh, PartitionSpec("core"))

    def prefetch(in_maps):
        concat_in = [
            _np.concatenate([_np.asarray(in_maps[c][nm])
                             for c in range(n_cores)], axis=0)
            for nm in in_names]
        return [jax.device_put(a, sharding) for a in concat_in]

    def run(args):
        if isinstance(args, list) and args and isinstance(args[0], dict):
            args = prefetch(args)
        concat_zeros = [
            _np.zeros((n_cores * z.shape[0], *z.shape[1:]), z.dtype)
            for z in zero_outs]
        out_arrs = sharded(*args, *concat_zeros)
        return [
            {nm: _np.asarray(out_arrs[i]).reshape(
                n_cores, *out_avals[i].shape)[c]
             for i, nm in enumerate(out_names)}
            for c in range(n_cores)]

    run.prefetch = prefetch
    return run


def _run(nc, in_maps):
    if id(nc) not in _RUN_CACHE:
        _RUN_CACHE[id(nc)] = _make_runner(nc)
    return _RUN_CACHE[id(nc)](in_maps)


def _prefetch(nc, in_maps):
    if id(nc) not in _RUN_CACHE:
        _RUN_CACHE[id(nc)] = _make_runner(nc)
    pf = _RUN_CACHE[id(nc)].prefetch(in_maps)
    for a in pf:
        a.block_until_ready()
    return pf


# ------------------------------------------------------------------ main entry
def kernel(node_feat, src, dst, graph_ids,
           W1, al1, ar1, b1, W2, al2, ar2, b2, W3, al3, ar3, b3,
           Wg1, bg1, Wg2, bg2, Wmu, bmu, Wlv, blv, cfg=None):
    cfg = cfg or CFG_FULL
    nc_ = cfg.ncores
    prep = host_prep(cfg, node_feat, src, dst, graph_ids)

    # pack weights
    wbmat = np.zeros((128, WBCOLS), BF16)
    wfmat = np.zeros((128, WFCOLS), np.float32)
    smb = np.zeros((1, SBCOLS), BF16)
    smf = np.zeros((1, SFCOLS), np.float32)

    def putb(nm, arr):
        o, ncols = SEG_B[nm]
        wbmat[:, o:o + ncols] = arr.reshape(128, ncols).astype(BF16)

    def putf(nm, arr):
        o, ncols = SEG_F[nm]
        wfmat[:, o:o + ncols] = arr.reshape(128, ncols).astype(np.float32)

    for li, (Wl, all_, arl) in enumerate([(W1, al1, ar1), (W2, al2, ar2),
                                          (W3, al3, ar3)]):
        Waug, Wr = fold_weights(np.asarray(Wl, np.float32),
                                np.asarray(all_, np.float32),
                                np.asarray(arl, np.float32))
        putb(f"W{li + 1}a", wstack(Waug))
        putb(f"Wr{li + 1}", wstack(Wr))
    putb("Wg1", wstack(np.asarray(Wg1, np.float32)))
    putf("Wmu", wstack(np.asarray(Wmu, np.float32)))
    putf("Wlv", wstack(np.asarray(Wlv, np.float32)))
    for nm, arr in [("b1", b1), ("b2", b2), ("b3", b3), ("Wg2", Wg2)]:
        o, ncols = SEG_SB[nm]
        smb[0, o:o + ncols] = np.asarray(arr, np.float32).reshape(-1).astype(BF16)
    for nm, arr in [("bg1", bg1), ("bmu", bmu), ("blv", blv), ("bg2", bg2)]:
        o, ncols = SEG_SF[nm]
        smf[0, o:o + ncols] = np.asarray(arr, np.float32).reshape(-1)

    SHARD = 128 // nc_
    ncF = _get(("FUSED",), lambda: build_fused(cfg))
    in_maps = [dict(
        x1T=prep["x1T"][c],
        idx=prep["idx32"][c],
        dpos=prep["dpos"][c],
        gid=prep["gid"][c],
        wb=np.ascontiguousarray(wbmat[c * SHARD:(c + 1) * SHARD]),
        wf=np.ascontiguousarray(wfmat[c * SHARD:(c + 1) * SHARD]),
        smb=smb, smf=smf) for c in range(nc_)]
    handle = _prefetch(ncF, in_maps)
    outs = _run(ncF, handle)
    mu = np.concatenate([outs[c]["mlv"][:cfg.gpc] for c in range(nc_)], 0)
    lv = np.concatenate([outs[c]["mlv"][cfg.gpad:cfg.gpad + cfg.gpc]
                         for c in range(nc_)], 0)
    return np.asarray(mu, np.float32), np.asarray(lv, np.float32)
